# revision 85
# baseline (speedup 1.0000x reference)
"""Trainium2 Bass kernel for a dense transformer block (pre-LN attn + MLP).

B=4, T=2048, D=768, H=12 (DH=64), DFF=3072, fp32.

Sharding: 8 cores = 4 batches x 2 roles. Each core processes one batch and
owns 1024 query tokens (two 512-blocks, paired {0,3}/{1,2} for causal load
balance). K/V are computed for the full 2048 tokens on both cores of a batch
(cheap), so there are NO collectives.

SPMD uniformity: all 8 cores run ONE identical NEFF. Causal structure is
carried in DATA, not code:
  - host permutes each batch's token axis to [own0, own1, otherA, otherB]
  - q-slot0 attends s-chunks {0..3, 8..11}; q-slot1 attends s-chunks {0..15}
  - per-(slot,chunk) exp scale/bias inputs select live / dead (zero) chunks
  - 4 canonical triangular masks handle the self-diagonal 512-blocks

Everything on-chip runs in a transposed layout (features on partitions,
tokens on the free axis) so no on-chip transposes are needed; all weight /
input transposes happen on the host in numpy. Matmuls run as float32r
(full PE speed, ~bf16x2 precision). LayerNorm statistics are computed with
ones-column matmuls; per-token stats are broadcast across partitions with
K=1 outer-product matmuls. Softmax denominators come for free from a ones
column appended to V (65-row PV matmul); the divide is folded in after PV.
"""

import sys

sys.path.insert(0, "/opt/trn_rl_repo")

from contextlib import ExitStack

import numpy as np

import concourse.bass as bass
import concourse.mybir as mybir
import concourse.tile as tile
from concourse import bacc
from concourse.bass_utils import run_bass_kernel_spmd

F32 = mybir.dt.float32
F32R = mybir.dt.float32r
AF = mybir.ActivationFunctionType
BF16 = mybir.dt.bfloat16
F8 = mybir.dt.float8e4
DR = mybir.MatmulPerfMode.DoubleRow
ALU = mybir.AluOpType

# fp8 pre-scales (host folds these into the weights; kernel divides out)
S_W1 = 16.0     # W1 stored as 16*W1
S_H1 = 16.0     # h1 stored as 16*relu(.) == relu(psum) directly (scale 1)
S_W2 = 16.0     # W2 stored as 16*W2
C_MLP = 1.0 / (S_H1 * S_W2)          # o2 -> true h@W2.T scale
S_YDIV = 1.0    # PV sums pre-scale before the divide (1 = none)
C_WO = 1.0      # wo-out descale folded into the residual add

H, D, DFF = 12, 768, 3072
DH = 64
B, T = 4, 2048
EPS = 1e-5
P = 128
NC = D // P          # 6 feature chunks
NF = DFF // P        # 24 ff tiles
TB = 512             # token block
NTB = T // TB        # 4 blocks
SLOT_CHUNKS = [[0, 1, 2, 3, 8, 9, 10, 11], list(range(16))]
# role -> permuted block order [own0, own1, restA, restB] (original block ids)
ROLE_ORDER = [[0, 3, 1, 2], [1, 2, 0, 3]]
DEAD = -30000.0      # exp(DEAD) == 0 in fp32

_cached = {}
PHASE_MARKS = []


def _mark(nc, name):
    PHASE_MARKS.append((name, nc.next_id()))


def _build_nc(use_be1=False, use_b1=False):
    nc = bacc.Bacc("TRN2", target_bir_lowering=False, debug=False,
                   enable_asserts=False, num_devices=8)

    def din(name, shape, dt=F32R):
        return nc.dram_tensor(name, shape, dt, kind="ExternalInput").ap()

    xt_d = din("xt", [D, T])                 # X[b].T, token-permuted (f32)
    xtb_d = din("xtb", [D, T], BF16)         # same, bf16 (QKV matmul input)
    wqt_d = din("wqt", [D, D], BF16)         # g1-folded w_q as [c, m]
    wkt_d = din("wkt", [D, D], BF16)
    wvt_d = din("wvt", [D, D], BF16)
    wo_d = din("wo", [D, D], BF16)           # natural [m, c]
    w1t_d = din("w1t", [NF // 4, D, 2, 4 * P], F8)  # [fgrp, c, (hi/lo), f]
    w2t_d = din("w2t", [DFF, 2, D], F8)      # 16*W2.T  [f, (hi/lo), c]
    ones512_d = din("ones512", [1, TB])      # ones row for b2 outer
    b2s_d = din("b2s", [1, D])               # 128*b2 as a row
    onesr_d = din("onesr", [1, P])           # outer-product lhsT
    onesc_d = din("onesc", [P, 1])           # column-sum lhsT (f32)
    onescb_d = din("onescb", [P, 1], BF16)   # column-sum lhsT (bf16)
    masks_d = din("masks", [4, P, TB], BF16)  # tri masks (shared by ph)
    scalein_d = din("scalein", [P, 24], F32) # exp scale per (slot,chunk)
    biasin_d = din("biasin", [P, 24], F32)   # exp bias per (slot,chunk)
    # LN1 is folded into the QKV path: -colsum(g1-folded w) rows (q, k, v)
    # for the rank-1 mean correction (and optional be1-fold rows)
    nws_d = din("nws", [3, D])
    if use_be1:
        wbe_d = din("wbe", [3, D])
    g2_d = din("g2v", [D], F32)
    be2_d = din("be2v", [D], F32)
    g2r_d = din("g2r", [1, D])
    b1_d = din("b1v", [DFF], F32)

    outt_d = nc.dram_tensor("outt", [D, 1024], F32, kind="ExternalOutput").ap()

    xt_r = xt_d.rearrange("(j p) t -> p j t", p=P)
    xtb_r = xtb_d.rearrange("(j p) t -> p j t", p=P)

    with tile.TileContext(nc) as tc, ExitStack() as ctx, \
         nc.allow_low_precision(reason="fp32r/bf16 intermediates are intended"):
        consts = ctx.enter_context(tc.tile_pool(name="consts", bufs=1))
        ps = ctx.enter_context(tc.tile_pool(name="ps", bufs=1, space="PSUM"))
        rows = ctx.enter_context(tc.tile_pool(name="rows", bufs=1))
        work = ctx.enter_context(tc.tile_pool(name="work", bufs=2))

        onesr_sb = consts.tile([1, P], F32R, tag="onesr")
        onesc_sb = consts.tile([P, 1], F32R, tag="onesc")
        onescb_sb = consts.tile([P, 1], BF16, tag="onescb")
        scale_sb = consts.tile([P, 24], F32, tag="scalein")
        bias_sb = consts.tile([P, 24], F32, tag="biasin")
        g2_sb = consts.tile([P, NC], F32, tag="g2")
        be2_sb = consts.tile([P, NC], F32, tag="be2")
        g2r_sb = consts.tile([1, D], F32R, tag="g2r")
        b1_sb = consts.tile([P, NF], F32, tag="b1")
        ones512_sb = consts.tile([1, TB], F32R, tag="ones512")
        b2s_sb = consts.tile([1, D], F32R, tag="b2s")
        # q/k/v correction rows (matmul operands must share base partition 0)
        nws_sb = {i: consts.tile([1, D], F32R, tag=f"nws{i}", name=f"nws{i}")
                  for i in range(3)}
        if use_be1:
            wbe_sb = {i: consts.tile([1, D], F32R, tag=f"wbe{i}",
                                     name=f"wbe{i}") for i in range(3)}

        def _early_const_dmas():
            nc.sync.dma_start(onesc_sb[:], onesc_d)
            nc.sync.dma_start(onescb_sb[:], onescb_d)
            nc.sync.dma_start(onesr_sb[:], onesr_d)
            for i in range(3):
                nc.sync.dma_start(nws_sb[i], nws_d[i:i + 1, :])
            if use_be1:
                for i in range(3):
                    nc.sync.dma_start(wbe_sb[i], wbe_d[i:i + 1, :])
                nc.sync.dma_start(ones512_sb[:], ones512_d)

        def _late_const_dmas():
            nc.sync.dma_start(scale_sb[:], scalein_d)
            nc.sync.dma_start(bias_sb[:], biasin_d)
            nc.sync.dma_start(g2r_sb[:], g2r_d)
            for sb, d in ((g2_sb, g2_d), (be2_sb, be2_d)):
                nc.sync.dma_start(sb[:], d.rearrange("(j p) -> p j", p=P))
            nc.sync.dma_start(b1_sb[:], b1_d.rearrange("(j p) -> p j", p=P))
            if not use_be1:
                nc.sync.dma_start(ones512_sb[:], ones512_d)
            nc.sync.dma_start(b2s_sb[:], b2s_d)
            nc.sync.dma_start(masks_sb[:], masks_d.rearrange("o p t -> p o t"))

        def _sums(src_sl, sq_engines, ones, sq_dt):
            """s1/s2 column-sum matmuls; squares cycled over sq_engines.
            `ones`/`sq_dt` must match src dtype class (32-bit vs not)."""
            s1 = ps.tile([1, TB], F32, tag="acc", bufs=2, name="s1")
            s2 = ps.tile([1, TB], F32, tag="acc", bufs=2, name="s2")
            for j in range(NC):
                nc.tensor.matmul(s1[:], ones[:], src_sl[:, j, :],
                                 start=(j == 0), stop=(j == NC - 1))
            for j in range(NC):
                sq = work.tile([P, TB], sq_dt, tag="sq", bufs=2)
                eng = sq_engines[j % len(sq_engines)]
                if eng is nc.scalar:
                    nc.scalar.activation(sq[:], src_sl[:, j, :], AF.Square)
                else:
                    eng.tensor_mul(sq[:], src_sl[:, j, :], src_sl[:, j, :])
                nc.tensor.matmul(s2[:], ones[:], sq[:],
                                 start=(j == 0), stop=(j == NC - 1))
            return s1, s2

        def _muvar(s1, s2):
            mu = rows.tile([1, TB], F32R, tag="mu", bufs=2)
            t = rows.tile([1, TB], F32R, tag="tmp", bufs=2)
            r = rows.tile([1, TB], F32R, tag="r", bufs=2)
            nc.vector.tensor_scalar_mul(mu[:], s1[:], 1.0 / D)
            nc.vector.tensor_mul(t[:], mu[:], mu[:])
            nc.vector.scalar_tensor_tensor(t[:], s2[:], 1.0 / D, t[:],
                                           ALU.mult, ALU.subtract)
            nc.vector.tensor_scalar_add(t[:], t[:], EPS)
            nc.scalar.activation(t[:], t[:], AF.Sqrt)
            nc.vector.reciprocal(r[:], t[:])
            return mu, r, t

        def ln_stats(src_sl):
            """src_sl: [128, NC, TB] slice. Returns (r, mur) rows in SBUF.
            Avoids ACT entirely (it is exp-saturated during attention)."""
            s1, s2 = _sums(src_sl, (nc.vector, nc.gpsimd), onesc_sb, F32R)
            mu, r, _ = _muvar(s1, s2)
            mur = rows.tile([1, TB], F32R, tag="mur", bufs=1)
            nc.vector.tensor_mul(mur[:], mu[:], r[:])
            return r, mur

        def ln1_stats(tb, xt_t):
            """Stats for the folded LN1: returns (mu, bcrs, rcol).

            mu: [1,TB] f32r row (rank-1 correction rhs); bcrs: [P,TB] bf16
            broadcast of 1/sigma (columns); rcol: [P,4] f32 1/sigma along
            partitions for this block's four 128-token s-chunks (V scale).
            """
            sqe = (nc.scalar, nc.vector, nc.gpsimd) if tb == 0 \
                else (nc.scalar, nc.gpsimd)
            s1, s2 = _sums(xt_t[:], sqe, onescb_sb, BF16)
            mu, r, sig = _muvar(s1, s2)
            bcr = ps.tile([P, TB], F32, tag="yt", bufs=2, name="bcs")
            nc.tensor.matmul(bcr[:], onesr_sb[:], r[:], start=True, stop=True)
            bcrs = work.tile([P, TB], BF16, tag="bcrs")
            nc.vector.tensor_copy(bcrs[:], bcr[:])
            rtp = ps.tile([P, TB], F32, tag="yt", bufs=2, name="rtp")
            for si in range(4):
                nc.tensor.matmul(rtp[:, 4 * si:4 * si + 4],
                                 r[:, si * P:(si + 1) * P],
                                 onesr_sb[:, 0:4], start=True, stop=True)
            rcol = work.tile([P, 4], F32, tag="rcol")
            nc.vector.tensor_copy(
                rcol[:], rtp[:, 0:16].rearrange("p (a b) -> p a b",
                                                b=4)[:, :, 0])
            return mu, bcrs, rcol, sig

        def gen_ln_normalize(src_sl, dst_sl, r, mur, g_sb, be_sb, g_row):
            """dst = ((src*g[p])*bc(r) + be[p]) - bc(g[p]*mur).
            Generator: yields after each PE matmul batch. The r-broadcast is
            copied to SBUF at once so its PSUM slot frees immediately."""
            bcr = ps.tile([P, TB], F32, tag="yt", bufs=2, name="bcs")[:]
            nc.tensor.matmul(bcr, onesr_sb[:], r[:], start=True, stop=True)
            bcrw = work.tile([P, TB], BF16, tag="bcrs", name="bcrw")
            nc.vector.tensor_copy(bcrw[:], bcr)
            yield
            for j in range(NC):
                bc2 = ps.tile([P, TB], F32, tag="acc", bufs=2, name="bc2")
                nc.tensor.matmul(bc2[:], g_row[:, j * P:(j + 1) * P], mur[:],
                                 start=True, stop=True)
                t1 = work.tile([P, TB], F32R, tag="sq", bufs=2, name="t1")
                nc.vector.scalar_tensor_tensor(t1[:], src_sl[:, j, :],
                                               g_sb[:, j:j + 1], bcrw[:],
                                               ALU.mult, ALU.mult)
                nc.vector.scalar_tensor_tensor(dst_sl[:, j, :], t1[:],
                                               be_sb[:, j:j + 1], bc2[:],
                                               ALU.add, ALU.subtract)
                yield

        # ---------------- Phase 1+2: LN1-folded QKV, software-pipelined -----
        _mark(nc, "ln1")
        es_yt = ExitStack()
        p_yt = es_yt.enter_context(tc.tile_pool(name="p_yt", bufs=2,
                                                side="right"))
        yt0 = p_yt.tile([P, NC, TB], BF16, tag="yt_all", name="yt0")
        yt1 = p_yt.tile([P, NC, TB], BF16, tag="yt_all", name="yt1")

        es_kqv = ExitStack()
        p_kqv = es_kqv.enter_context(tc.tile_pool(name="p_kqv", bufs=1,
                                                  side="right"))
        kt_sb = p_kqv.tile([P, NC, T], BF16, tag="kt")      # K^T [m, s]
        qt_sb = p_kqv.tile([P, NC, 1024], BF16, tag="qt")   # Q^T [m, t_own]
        v_sb = p_kqv.tile([P, 16, H * 65], BF16, tag="v")   # V_ext [s, (h,65)]
        v_view = v_sb.rearrange("p s (h e) -> p s h e", e=65)
        nc.vector.memset(v_view[:, :, :, 64:65], 1.0)

        es_masks = ExitStack()
        p_masks = es_masks.enter_context(tc.tile_pool(name="p_masks", bufs=1,
                                                      side="right"))
        p_e = es_masks.enter_context(tc.tile_pool(name="p_e", bufs=4,
                                                  side="right"))
        masks_sb = p_masks.tile([P, 4, TB], BF16, tag="masks")
        # tri chunks (di>0) exp/mask/PV all operate only on the live column
        # sub-range [lo:], so the masked-out columns are never read and need
        # no pre-zeroed tiles.

        es_wqkv = ExitStack()
        p_wqkv = es_wqkv.enter_context(tc.tile_pool(name="p_wqkv", bufs=1,
                                                    side="right"))
        wq_sb = p_wqkv.tile([P, NC, D], BF16, tag="wq")
        wk_sb = p_wqkv.tile([P, NC, D], BF16, tag="wk")
        wv_sb = p_wqkv.tile([P, NC, D], BF16, tag="wv")

        def qkv_for_tb(tb, xt_t, mu, bcrs, rcol, sig):
            """QKV projections straight from raw x^T with LN1 folded in:
            psum accumulates (g1-folded w)@x plus the rank-1 -colsum(w)*mu
            correction; the per-token 1/sigma lands at psum readout (bcrs
            columns for K/Q, rcol per-partition scale for V).
            Accumulators live two-per [P, 2*TB] PSUM tile on the "st" tag
            (idle during phase 1)."""
            tsl = slice(tb * TB, (tb + 1) * TB)
            _mark(nc, "qkv")

            def proj_pair(w_sb, ws_row, wbe_row, dst_sb, mtp):
                acc2 = ps.tile([P, 2 * TB], F32, tag="st", bufs=2, name="ka")
                for half in range(2):
                    mt = 2 * mtp + half
                    msl = slice(mt * P, (mt + 1) * P)
                    seg = acc2[:, half * TB:(half + 1) * TB]
                    for j in range(NC):
                        nc.tensor.matmul(seg, w_sb[:, j, msl], xt_t[:, j, :],
                                         start=(j == 0), stop=False)
                    nc.tensor.matmul(seg, ws_row[:, msl], mu[:],
                                     start=False, stop=(not use_be1))
                    if use_be1:
                        # be-term must survive the *1/sigma readout: rhs=sigma
                        nc.tensor.matmul(seg, wbe_row[:, msl], sig[:],
                                         start=False, stop=True)
                for half in range(2):
                    mt = 2 * mtp + half
                    seg = acc2[:, half * TB:(half + 1) * TB]
                    nc.vector.tensor_mul(dst_sb[:, mt, tsl], seg, bcrs[:])

            for mtp in range(3):
                proj_pair(wk_sb, nws_sb[1],
                          wbe_sb[1] if use_be1 else None, kt_sb, mtp)
            for si in range(4):
                st = tb * 4 + si
                lsl = slice(si * P, (si + 1) * P)
                acc2 = ps.tile([P, 2 * TB], F32, tag="st", bufs=2, name="va")
                for half, fsl, off, w in ((0, slice(0, TB), 0, TB),
                                          (1, slice(TB, D), TB, D - TB)):
                    seg = acc2[:, off:off + w]
                    for j in range(NC):
                        nc.tensor.matmul(seg, xt_t[:, j, lsl],
                                         wv_sb[:, j, fsl],
                                         start=(j == 0), stop=False)
                    nc.tensor.matmul(seg, mu[:, lsl], nws_sb[2][:, fsl],
                                     start=False, stop=(not use_be1))
                    if use_be1:
                        nc.tensor.matmul(seg, sig[:, lsl], wbe_sb[2][:, fsl],
                                         start=False, stop=True)
                for half, off, w in ((0, 0, TB), (1, TB, D - TB)):
                    src = acc2[:, off:off + w].rearrange(
                        "p (h e) -> p h e", e=64)
                    h0 = half * 8
                    nc.scalar.activation(
                        v_view[:, st, h0:h0 + w // 64, 0:64], src, AF.Copy,
                        scale=rcol[:, si:si + 1])
            if tb < 2:
                for mtp in range(3):
                    proj_pair(wq_sb, nws_sb[0],
                              wbe_sb[0] if use_be1 else None, qt_sb, mtp)

        def attn_division(pend):
            # recip + broadcast + divide from the SBUF copies; deferred into
            # the NEXT mt's chunk stream so the bc matmuls never head-of-line
            # block the next mt's QKs
            yt_all, mt, yt_sbs = pend
            for ph in range(2):
                o = ph * 64
                yt_sb = yt_sbs[ph]
                rc = rows.tile([1, TB], F32R, tag="rc", bufs=2)
                nc.vector.reciprocal(rc[:], yt_sb[64:65, :])
                bc = ps.tile([64, TB], F32, tag="acc", bufs=2, name="abc")
                nc.tensor.matmul(bc[:], onesr_sb[:, 0:64],
                                 rc[:], start=True, stop=True)
                nc.vector.tensor_mul(yt_all[o:o + 64, mt, :],
                                     yt_sb[0:64, :], bc[:])

        def attn_mt(sl_i, yt_all, mt, fill, pend):
            chunks = SLOT_CHUNKS[sl_i]
            yt2 = [ps.tile([65, TB], F32, tag="yt", bufs=2,
                           name=f"yt_{sl_i}_{mt}_{ph}") for ph in range(2)]
            for ci, ch in enumerate(chunks):
                sb_idx = (0 if sl_i == 0 else 8) + ci
                di = ch - 4 * sl_i
                tri = 0 <= di < 4
                lo = 128 * di if tri else 0
                st2 = ps.tile([P, 2 * TB], F32, tag="st", bufs=2)
                qsub = slice(sl_i * TB + lo, (sl_i + 1) * TB)
                for ph in range(2):
                    o = ph * 64
                    nc.tensor.matmul(
                        st2[:, ph * TB + lo:(ph + 1) * TB],
                        kt_sb[o:o + 64, mt, ch * P:(ch + 1) * P],
                        qt_sb[o:o + 64, mt, qsub],
                        start=True, stop=True)
                e_tile = p_e.tile([P, 2 * TB], BF16, tag="e",
                                  name=f"e_{sl_i}_{mt}_{ci}")
                e_sb = e_tile[:]
                ev2 = e_sb.rearrange("p (two t) -> p two t", two=2)
                if lo:
                    ev = ev2[:, :, lo:]
                    sv = st2[:].rearrange("p (two t) -> p two t",
                                          two=2)[:, :, lo:]
                else:
                    ev, sv = e_sb, st2[:]
                nc.scalar.activation(
                    ev, sv, AF.Exp,
                    bias=bias_sb[:, sb_idx:sb_idx + 1],
                    scale=scale_sb[:, sb_idx:sb_idx + 1])
                if tri:
                    for ph in range(2):
                        nc.vector.tensor_mul(ev2[:, ph, lo:], ev2[:, ph, lo:],
                                             masks_sb[:, di, lo:])
                for ph in range(2):
                    h = 2 * mt + ph
                    nc.tensor.matmul(
                        yt2[ph][:, lo:], v_sb[:, ch, h * 65:(h + 1) * 65],
                        ev2[:, ph, lo:],
                        start=(ci == 0),
                        stop=(ci == len(chunks) - 1))
                if ci == 1 and pend is not None:
                    attn_division(pend)
                    pend = None
                if fill is not None:
                    next(fill, None)
            yt_sbs = []
            for ph in range(2):
                # copy [65,TB] to SBUF immediately: frees the PSUM bank so
                # the next mt's PV can start during the division
                yt_sb = work.tile([65, TB], F32R, tag="ydiv", bufs=3)
                nc.vector.tensor_copy(yt_sb[:], yt2[ph][:])
                yt_sbs.append(yt_sb)
            return (yt_all, mt, yt_sbs)


        with tc.tile_pool(name="p_xtr", bufs=3) as p_xtr:
            stats = {}
            xts = {}
            for tb in range(NTB):
                tsl = slice(tb * TB, (tb + 1) * TB)
                xt_t = p_xtr.tile([P, NC, TB], BF16, tag="xtr")
                if tb == 0:
                    # tiny consts first (s1's onesc lhsT must not queue
                    # behind the bulk transfers), then the first block split
                    # so the stats tree can start after ~1/3 of it
                    _early_const_dmas()
                    for jj in range(3):
                        nc.sync.dma_start(xt_t[:, 2 * jj:2 * jj + 2, :],
                                          xtb_r[:, 2 * jj:2 * jj + 2, tsl])
                else:
                    nc.sync.dma_start(xt_t[:], xtb_r[:, :, tsl])
                if tb == 0:
                    nc.sync.dma_start(wk_sb[:],
                                      wkt_d.rearrange("(j p) m -> p j m", p=P))
                    nc.sync.dma_start(wv_sb[:],
                                      wvt_d.rearrange("(j p) m -> p j m", p=P))
                    nc.sync.dma_start(wq_sb[:],
                                      wqt_d.rearrange("(j p) m -> p j m", p=P))
                if tb == 1:
                    _late_const_dmas()
                xts[tb] = xt_t
                # emit qkv(tb-1) before stats(tb): the bulk PE work is ready
                # to run, so the in-order PE stream never parks on the
                # square/sum chain of the next block
                if tb > 0:
                    qkv_for_tb(tb - 1, xts[tb - 1][:], *stats[tb - 1])
                stats[tb] = ln1_stats(tb, xt_t)
            # slot0's first two mt's only need tb0/tb2 projections: run them
            # before qkv(3) so their exps fill phase-1's idle ACT while the
            # last projection block grinds on PE
            _mark(nc, "attn")
            pend0 = attn_mt(0, yt0, 0, None, None)
            pend0 = attn_mt(0, yt0, 1, None, pend0)
            qkv_for_tb(NTB - 1, xts[NTB - 1][:], *stats[NTB - 1])
        es_wqkv.close()

        # ---------------- Phase 3: attention (+ per-slot wo/LN2) -----------
        # Slot 1's chunk loop is exp(ACT)-bound with ~190ns/chunk of PE
        # slack, so slot 0's w_o projection, LN2 and the first MLP1 groups
        # are threaded through it as fine-grained fillers (one small PE
        # batch per attention chunk).
        _mark(nc, "attn")
        w1t_r = w1t_d.rearrange("g (j p) two f -> p g j two f", p=P)
        w2t_r = w2t_d.rearrange("(f p) two c -> p f two c", p=P)
        outt_r = outt_d.rearrange("(j p) t -> p j t", p=P)
        p_xp = ctx.enter_context(tc.tile_pool(name="p_xp", bufs=1))
        xp_sb = p_xp.tile([P, NC, 1024], F32R, tag="xp")
        p_xn2 = ctx.enter_context(tc.tile_pool(name="p_xn2", bufs=1))
        xn2_sb = p_xn2.tile([P, NC, 1024], F8, tag="xn2")
        p_h1a = ctx.enter_context(tc.tile_pool(name="p_h1a", bufs=1))
        p_wmlp = ctx.enter_context(tc.tile_pool(name="p_wmlp", bufs=2))
        h1s = {0: p_h1a.tile([P, NF, TB], F8, tag="h1a", name="h1a"),
               1: None}
        es_wo = ExitStack()
        p_wo = es_wo.enter_context(tc.tile_pool(name="p_wo", bufs=1))
        wo_sb = p_wo.tile([P, NC, D], BF16, tag="wo")
        nc.sync.dma_start(wo_sb[:], wo_d.rearrange("(j p) m -> p j m", p=P))
        xo0_sb = p_wo.tile([P, NC, TB], BF16, tag="xo", bufs=1, name="xo0")
        nc.sync.dma_start(xo0_sb[:], xtb_r[:, :, 0:TB])
        xo_hold = {0: xo0_sb}

        def gen_wo(sl_i, yt_all):
            _mark(nc, "wo")
            qsl = slice(sl_i * TB, (sl_i + 1) * TB)
            xo_t = xo_hold[sl_i]
            for ct in range(NC):
                ao = ps.tile([P, TB], F32, tag="acc", bufs=2, name="ao")
                for mc in range(NC):
                    nc.tensor.matmul(ao[:],
                                     wo_sb[:, mc, ct * P:(ct + 1) * P],
                                     yt_all[:, mc, :],
                                     start=(mc == 0), stop=(mc == NC - 1))
                    if mc % 2:
                        yield
                nc.vector.tensor_add(xp_sb[:, ct, qsl], xo_t[:, ct, :],
                                     ao[:])

        def gen_xo1():
            # slot1 residual load: reuses xo0's slot, so it must be emitted
            # only after all of gen_wo(0)'s reads
            xo1 = p_wo.tile([P, NC, TB], BF16, tag="xo", bufs=1, name="xo1")
            nc.sync.dma_start(xo1[:], xtb_r[:, :, TB:2 * TB])
            xo_hold[1] = xo1
            return
            yield

        def gen_ln2(sl_i):
            _mark(nc, "ln2")
            qsl = slice(sl_i * TB, (sl_i + 1) * TB)
            src = xp_sb[:, :, qsl]
            s1 = ps.tile([1, TB], F32, tag="acc", bufs=2, name="s1")
            s2 = ps.tile([1, TB], F32, tag="acc", bufs=2, name="s2")
            for j in range(NC):
                nc.tensor.matmul(s1[:], onesc_sb[:], src[:, j, :],
                                 start=(j == 0), stop=(j == NC - 1))
                if j % 2:
                    yield
            sqe = (nc.vector, nc.gpsimd)
            for j in range(NC):
                sq = work.tile([P, TB], F32R, tag="sq", bufs=2)
                sqe[j % 2].tensor_mul(sq[:], src[:, j, :], src[:, j, :])
                nc.tensor.matmul(s2[:], onesc_sb[:], sq[:],
                                 start=(j == 0), stop=(j == NC - 1))
                if j % 2:
                    yield
            mu2, r2, _ = _muvar(s1, s2)
            mur2 = rows.tile([1, TB], F32R, tag="mur", bufs=1)
            nc.vector.tensor_mul(mur2[:], mu2[:], r2[:])
            yield
            yield from gen_ln_normalize(src, xn2_sb[:, :, qsl], r2, mur2,
                                        g2_sb, be2_sb, g2r_sb)

        def gen_mlp1(groups, tbs, after_group=None):
            _mark(nc, "mlp1")
            for ft4 in groups:
                w1_t = p_wmlp.tile([P, NC, 2, 4 * P], F8, tag="w1", bufs=2)
                nc.sync.dma_start(w1_t[:], w1t_r[:, ft4])
                for sub in range(4):
                    ft = 4 * ft4 + sub
                    for tb in tbs:
                        tsl = slice(tb * TB, (tb + 1) * TB)
                        hp = ps.tile([P, TB], F32, tag="acc", bufs=2,
                                     name="hp")
                        for j in range(NC):
                            nc.tensor.matmul(
                                hp[:],
                                w1_t[:, j, :, sub * P:(sub + 1) * P],
                                xn2_sb[:, j, tsl].unsqueeze(1)
                                .broadcast_to([P, 2, TB]),
                                start=(j == 0), stop=(j == NC - 1),
                                perf_mode=DR)
                            if j % 3 == 2:
                                yield
                        # h1 = relu(psum + S_H1*b1) (S_H1 == S_W1 so scale=1)
                        if use_b1:
                            nc.scalar.activation(h1s[tb][:, ft, :], hp[:],
                                                 AF.Relu,
                                                 bias=b1_sb[:, ft:ft + 1],
                                                 scale=S_H1 / S_W1)
                        else:
                            nc.vector.tensor_scalar_max(h1s[tb][:, ft, :],
                                                        hp[:], 0.0)
                        yield
                if after_group is not None:
                    after_group(ft4)

        # slot 0 rest (mts 0-1 already ran inside phase 1)
        for mt in range(2, NC):
            pend0 = attn_mt(0, yt0, mt, None, pend0)
        # slot 1 with interleaved fillers
        import itertools as _it
        fill = _it.chain(gen_wo(0, yt0), gen_xo1(), gen_ln2(0),
                         gen_mlp1(range(0, 3), (0,)))
        for mt in range(NC):
            pend0 = attn_mt(1, yt1, mt, fill, pend0)
        attn_division(pend0)
        for _ in fill:
            pass

        es_masks.close()
        es_kqv.close()

        # prefetch the first tail-MLP1 w1 groups so their transfers run
        # under wo(slot1)/LN2(slot1)
        tail_items = [(3, 0), (4, 0), (5, 0)] + [(g, 1) for g in range(6)]
        w1_tiles = {}

        def _issue_w1(i):
            if i < len(tail_items):
                t = p_wmlp.tile([P, NC, 2, 4 * P], F8, tag="w1", bufs=2,
                                name=f"w1b_{i}")
                nc.sync.dma_start(t[:], w1t_r[:, tail_items[i][0]])
                w1_tiles[i] = t

        for i in range(2):
            _issue_w1(i)
        for _ in gen_wo(1, yt1):
            pass
        for _ in gen_ln2(1):
            pass

        es_wo.close()
        es_yt.close()

        # ---------------- Phase 6: MLP tail ----------------
        _mark(nc, "mlp")
        with tc.tile_pool(name="p_h1b", bufs=1) as p_h1b, \
             tc.tile_pool(name="p_w2", bufs=1) as p_w2, \
             tc.tile_pool(name="p_out", bufs=4) as p_out:
            h1s[1] = p_h1b.tile([P, NF, TB], F8, tag="h1b", name="h1b")
            w2_sb = p_w2.tile([P, NF, 2, D], F8, tag="w2full")
            # rest of MLP1 as a depth-3 w1-prefetch pipeline; w2 streams in
            # behind the w1 loads
            for i, (ft4, tb) in enumerate(tail_items):
                w1_t = w1_tiles.pop(i)
                tsl = slice(tb * TB, (tb + 1) * TB)
                for sub in range(4):
                    ft = 4 * ft4 + sub
                    hp = ps.tile([P, TB], F32, tag="acc", bufs=2, name="hp")
                    for j in range(NC):
                        nc.tensor.matmul(
                            hp[:], w1_t[:, j, :, sub * P:(sub + 1) * P],
                            xn2_sb[:, j, tsl].unsqueeze(1)
                            .broadcast_to([P, 2, TB]),
                            start=(j == 0), stop=(j == NC - 1),
                            perf_mode=DR)
                    nc.scalar.activation(h1s[tb][:, ft, :], hp[:], AF.Relu,
                                         bias=b1_sb[:, ft:ft + 1],
                                         scale=S_H1 / S_W1)
                _issue_w1(i + 2)
                if i < 4:
                    nc.sync.dma_start(w2_sb[:, 6 * i:6 * i + 6],
                                      w2t_r[:, 6 * i:6 * i + 6, :, :])
            # MLP2: uneven 4+2 column grouping so the last-finishing group
            # has only 4 output DMAs in the tail
            for cts in (range(0, 4), range(4, 6)):
                cts = list(cts)
                o2s = {}
                for idx, ct in enumerate(cts):
                    if idx < 2:
                        tag = ("acc", "yt")[idx]
                        for tb in range(2):
                            o2s[(ct, tb)] = ps.tile(
                                [P, TB], F32, tag=tag, bufs=2,
                                name=f"o2_{ct}_{tb}")
                    else:
                        stp = ps.tile([P, 2 * TB], F32, tag="st", bufs=2,
                                      name=f"o2st_{ct}")
                        o2s[(ct, 0)] = stp[:, 0:TB]
                        o2s[(ct, 1)] = stp[:, TB:2 * TB]
                # seed each accumulator with (b2/C_MLP) x ones
                for ct in cts:
                    for tb in range(2):
                        nc.tensor.matmul(o2s[(ct, tb)][:],
                                         b2s_sb[:, ct * P:(ct + 1) * P],
                                         ones512_sb[:],
                                         start=True, stop=False)
                for ft in range(NF):
                    for tb in range(2):
                        rhs = h1s[tb][:, ft, :].unsqueeze(1).broadcast_to(
                            [P, 2, TB])
                        for ct in cts:
                            nc.tensor.matmul(
                                o2s[(ct, tb)][:],
                                w2_sb[:, ft, :, ct * P:(ct + 1) * P],
                                rhs, start=False, stop=(ft == NF - 1),
                                perf_mode=DR)
                for tb in range(2):
                    tsl = slice(tb * TB, (tb + 1) * TB)
                    for ct in cts:
                        ot = p_out.tile([P, TB], F32, tag="ot",
                                        name=f"ot_{ct}_{tb}")
                        nc.vector.scalar_tensor_tensor(
                            ot[:], o2s[(ct, tb)][:], C_MLP,
                            xp_sb[:, ct, tsl], ALU.mult, ALU.add)
                        nc.sync.dma_start(outt_r[:, ct, tsl], ot[:])

    nc.compile()
    return nc


def _hilo(w, f8):
    """[..., n] -> [..., 2, n] fp8 (hi, residual-lo) planes."""
    hi = w.astype(f8)
    lo = (w - hi.astype(np.float32)).astype(f8)
    return np.ascontiguousarray(np.stack([hi, lo], axis=-2))


def _host_inputs(X, w_q, w_k, w_v, w_o, W1, b1, W2, b2, g1, be1, g2, be2):
    """Build the 8 per-core input dicts."""
    f32 = np.float32
    import ml_dtypes as _mld
    _f8 = _mld.float8_e4m3
    _bf = _mld.bfloat16
    g1v = np.asarray(g1, f32)
    be1v = np.asarray(be1, f32)
    # LN1 fold: g1 into the QKV weight columns; mean correction rows are the
    # negated column sums; optional be1 rows handle a nonzero LN1 shift
    wqg = np.asarray(w_q, f32).reshape(D, D) * g1v[None, :]
    wkg = np.asarray(w_k, f32).reshape(D, D) * g1v[None, :]
    wvg = np.asarray(w_v, f32).reshape(D, D) * g1v[None, :]
    wqt = np.ascontiguousarray(wqg.T.astype(_bf))
    wkt = np.ascontiguousarray(wkg.T.astype(_bf))
    wvt = np.ascontiguousarray(wvg.T.astype(_bf))
    nws = np.ascontiguousarray(np.stack(
        [-wqg.sum(axis=1), -wkg.sum(axis=1), -wvg.sum(axis=1)]).astype(f32))
    use_be1 = bool(np.any(be1v))
    wo = np.ascontiguousarray(np.asarray(w_o, f32).astype(_bf))
    w1t = None  # bf16, set below
    w2t = None  # bf16, set below
    onesr = np.ones((1, P), f32)
    onesc = np.ones((P, 1), f32)
    onescb = np.ones((P, 1), _bf)
    onesv = None  # set below after bf16 import
    # 4 canonical self-diagonal masks: mask[k][s, t] = (128k + s <= t)
    import ml_dtypes
    bf16 = ml_dtypes.bfloat16
    masks = np.zeros((4, P, TB), bf16)
    ar_s = np.arange(P)[:, None]
    ar_t = np.arange(TB)[None, :]
    for k in range(4):
        masks[k] = (128 * k + ar_s <= ar_t).astype(bf16)
    w1t = _hilo(np.asarray(W1, f32).T * S_W1, _f8)   # [D, 2, DFF]
    w1t = np.ascontiguousarray(
        w1t.reshape(D, 2, NF // 4, 4 * P).transpose(2, 0, 1, 3))
    w2t = _hilo(np.asarray(W2, f32).T * S_W2, _f8)

    # per-role exp scale/bias: 24 = 8 (slot0) + 16 (slot1) chunk positions
    sc = {}
    bi = {}
    for role in range(2):
        order = ROLE_ORDER[role]
        s = np.full((24,), 0.125, f32)
        b = np.zeros((24,), f32)
        for sl_i in range(2):
            own_blk = order[sl_i]
            for ci, ch in enumerate(SLOT_CHUNKS[sl_i]):
                idx = (0 if sl_i == 0 else 8) + ci
                pos = ch // 4           # permuted 512-block of this s-chunk
                blk = order[pos]
                if pos == sl_i or blk < own_blk:
                    pass                # diagonal (tri-masked) or past: live
                else:
                    s[idx] = 0.0        # future: dead
                    b[idx] = DEAD
        sc[role] = np.broadcast_to(s, (P, 24)).copy()
        bi[role] = np.broadcast_to(b, (P, 24)).copy()

    g2r = np.asarray(g2, f32).reshape(1, D)
    shared = dict(wqt=wqt, wkt=wkt, wvt=wvt, wo=wo, w1t=w1t, w2t=w2t,
                  g2r=g2r, nws=nws,
                  onesr=onesr, onesc=onesc, onescb=onescb, masks=masks,
                  g2v=np.asarray(g2, f32), be2v=np.asarray(be2, f32),
                  b1v=np.asarray(b1, f32) * S_H1,
                  b2s=np.asarray(b2, f32).reshape(1, D) * (S_H1 * S_W2),
                  ones512=np.ones((1, TB), f32))
    if use_be1:
        shared["wbe"] = np.ascontiguousarray(np.stack(
            [np.asarray(w, f32).reshape(D, D) @ be1v
             for w in (w_q, w_k, w_v)]).astype(f32))

    in_maps = []
    for core in range(8):
        role, b_idx = core // 4, core % 4
        order = ROLE_ORDER[role]
        xb = np.asarray(X[b_idx], f32)          # [T, D]
        xperm = np.concatenate([xb[o * TB:(o + 1) * TB] for o in order], axis=0)
        xt = np.ascontiguousarray(xperm.T)      # [D, T]
        m = dict(shared)
        m["xt"] = xt
        m["xtb"] = np.ascontiguousarray(xt.astype(_bf))
        m["scalein"] = sc[role]
        m["biasin"] = bi[role]
        in_maps.append(m)
    return in_maps


def _assemble(results, dtype):
    out = np.empty((B, T, D), dtype)
    for core in range(8):
        role, b_idx = core // 4, core % 4
        order = ROLE_ORDER[role]
        ot = results[core]["outt"]              # [D, 1024]
        for sl_i in range(2):
            blk = order[sl_i]
            out[b_idx, blk * TB:(blk + 1) * TB] = \
                ot[:, sl_i * TB:(sl_i + 1) * TB].T
    return out


def kernel(X, w_q, w_k, w_v, w_o, W1, b1, W2, b2, g1, be1, g2, be2,
           _want_results=False, _trace=False):
    use_be1 = bool(np.any(np.asarray(be1)))
    use_b1 = bool(np.any(np.asarray(b1)))
    key = ("nc", use_be1, use_b1)
    if key not in _cached:
        _cached[key] = _build_nc(use_be1=use_be1, use_b1=use_b1)
        _cached["nc"] = _cached[key]
    nc = _cached[key]
    in_maps = _host_inputs(X, w_q, w_k, w_v, w_o, W1, b1, W2, b2,
                           g1, be1, g2, be2)
    res = run_bass_kernel_spmd(nc, in_maps, core_ids=list(range(8)),
                               trace=_trace)
    out = _assemble(res.results, np.asarray(X).dtype)
    if _want_results:
        return out, res
    return out



# revision 87
# speedup vs baseline: 1.0060x; 1.0060x over previous
"""Trainium2 Bass kernel for a dense transformer block (pre-LN attn + MLP).

B=4, T=2048, D=768, H=12 (DH=64), DFF=3072, fp32.

Sharding: 8 cores = 4 batches x 2 roles. Each core processes one batch and
owns 1024 query tokens (two 512-blocks, paired {0,3}/{1,2} for causal load
balance). K/V are computed for the full 2048 tokens on both cores of a batch
(cheap), so there are NO collectives.

SPMD uniformity: all 8 cores run ONE identical NEFF. Causal structure is
carried in DATA, not code:
  - host permutes each batch's token axis to [own0, own1, otherA, otherB]
  - q-slot0 attends s-chunks {0..3, 8..11}; q-slot1 attends s-chunks {0..15}
  - per-(slot,chunk) exp scale/bias inputs select live / dead (zero) chunks
  - 4 canonical triangular masks handle the self-diagonal 512-blocks

Everything on-chip runs in a transposed layout (features on partitions,
tokens on the free axis) so no on-chip transposes are needed; all weight /
input transposes happen on the host in numpy. Matmuls run as float32r
(full PE speed, ~bf16x2 precision). LayerNorm statistics are computed with
ones-column matmuls; per-token stats are broadcast across partitions with
K=1 outer-product matmuls. Softmax denominators come for free from a ones
column appended to V (65-row PV matmul); the divide is folded in after PV.
"""

import sys

sys.path.insert(0, "/opt/trn_rl_repo")

from contextlib import ExitStack

import numpy as np

import concourse.bass as bass
import concourse.mybir as mybir
import concourse.tile as tile
from concourse import bacc
from concourse.bass_utils import run_bass_kernel_spmd

F32 = mybir.dt.float32
F32R = mybir.dt.float32r
AF = mybir.ActivationFunctionType
BF16 = mybir.dt.bfloat16
F8 = mybir.dt.float8e4
DR = mybir.MatmulPerfMode.DoubleRow
ALU = mybir.AluOpType

# fp8 pre-scales (host folds these into the weights; kernel divides out)
S_W1 = 16.0     # W1 stored as 16*W1
S_H1 = 16.0     # h1 stored as 16*relu(.) == relu(psum) directly (scale 1)
S_W2 = 16.0     # W2 stored as 16*W2
C_MLP = 1.0 / (S_H1 * S_W2)          # o2 -> true h@W2.T scale
S_YDIV = 1.0    # PV sums pre-scale before the divide (1 = none)
C_WO = 1.0      # wo-out descale folded into the residual add

H, D, DFF = 12, 768, 3072
DH = 64
B, T = 4, 2048
EPS = 1e-5
P = 128
NC = D // P          # 6 feature chunks
NF = DFF // P        # 24 ff tiles
TB = 512             # token block
NTB = T // TB        # 4 blocks
SLOT_CHUNKS = [[0, 1, 2, 3, 8, 9, 10, 11], list(range(16))]
# role -> permuted block order [own0, own1, restA, restB] (original block ids)
ROLE_ORDER = [[0, 3, 1, 2], [1, 2, 0, 3]]
DEAD = -30000.0      # exp(DEAD) == 0 in fp32

_cached = {}
PHASE_MARKS = []


def _mark(nc, name):
    PHASE_MARKS.append((name, nc.next_id()))


def _build_nc(use_be1=False, use_b1=False):
    nc = bacc.Bacc("TRN2", target_bir_lowering=False, debug=False,
                   enable_asserts=False, num_devices=8)

    def din(name, shape, dt=F32R):
        return nc.dram_tensor(name, shape, dt, kind="ExternalInput").ap()

    xt_d = din("xt", [D, T])                 # X[b].T, token-permuted (f32)
    xtb_d = din("xtb", [D, T], BF16)         # same, bf16 (QKV matmul input)
    wqt_d = din("wqt", [D, D], BF16)         # g1-folded w_q as [c, m]
    wkt_d = din("wkt", [D, D], BF16)
    wvt_d = din("wvt", [D, D], BF16)
    wo_d = din("wo", [D, D], BF16)           # natural [m, c]
    w1t_d = din("w1t", [NF // 4, D, 2, 4 * P], F8)  # [fgrp, c, (hi/lo), f]
    w2t_d = din("w2t", [DFF, 2, D], F8)      # 16*W2.T  [f, (hi/lo), c]
    ones512_d = din("ones512", [1, TB])      # ones row for b2 outer
    b2s_d = din("b2s", [1, D])               # 128*b2 as a row
    onesr_d = din("onesr", [1, P])           # outer-product lhsT
    onesc_d = din("onesc", [P, 1])           # column-sum lhsT (f32)
    onescb_d = din("onescb", [P, 1], BF16)   # column-sum lhsT (bf16)
    masks_d = din("masks", [4, P, TB], BF16)  # tri masks (shared by ph)
    scalein_d = din("scalein", [P, 24], F32) # exp scale per (slot,chunk)
    biasin_d = din("biasin", [P, 24], F32)   # exp bias per (slot,chunk)
    # LN1 is folded into the QKV path: -colsum(g1-folded w) rows (q, k, v)
    # for the rank-1 mean correction (and optional be1-fold rows)
    nws_d = din("nws", [3, D])
    if use_be1:
        wbe_d = din("wbe", [3, D])
    g2_d = din("g2v", [D], F32)
    be2_d = din("be2v", [D], F32)
    g2r_d = din("g2r", [1, D])
    b1_d = din("b1v", [DFF], F32)

    outt_d = nc.dram_tensor("outt", [D, 1024], F32, kind="ExternalOutput").ap()

    xt_r = xt_d.rearrange("(j p) t -> p j t", p=P)
    xtb_r = xtb_d.rearrange("(j p) t -> p j t", p=P)

    with tile.TileContext(nc) as tc, ExitStack() as ctx, \
         nc.allow_low_precision(reason="fp32r/bf16 intermediates are intended"):
        consts = ctx.enter_context(tc.tile_pool(name="consts", bufs=1))
        ps = ctx.enter_context(tc.tile_pool(name="ps", bufs=1, space="PSUM"))
        rows = ctx.enter_context(tc.tile_pool(name="rows", bufs=1))
        work = ctx.enter_context(tc.tile_pool(name="work", bufs=2))

        onesr_sb = consts.tile([1, P], F32R, tag="onesr")
        onesc_sb = consts.tile([P, 1], F32R, tag="onesc")
        onescb_sb = consts.tile([P, 1], BF16, tag="onescb")
        scale_sb = consts.tile([P, 24], F32, tag="scalein")
        bias_sb = consts.tile([P, 24], F32, tag="biasin")
        g2_sb = consts.tile([P, NC], F32, tag="g2")
        be2_sb = consts.tile([P, NC], F32, tag="be2")
        g2r_sb = consts.tile([1, D], F32R, tag="g2r")
        b1_sb = consts.tile([P, NF], F32, tag="b1")
        ones512_sb = consts.tile([1, TB], F32R, tag="ones512")
        b2s_sb = consts.tile([1, D], F32R, tag="b2s")
        # q/k/v correction rows (matmul operands must share base partition 0)
        nws_sb = {i: consts.tile([1, D], F32R, tag=f"nws{i}", name=f"nws{i}")
                  for i in range(3)}
        if use_be1:
            wbe_sb = {i: consts.tile([1, D], F32R, tag=f"wbe{i}",
                                     name=f"wbe{i}") for i in range(3)}

        def _early_const_dmas():
            nc.sync.dma_start(onesc_sb[:], onesc_d)
            nc.sync.dma_start(onescb_sb[:], onescb_d)
            nc.sync.dma_start(onesr_sb[:], onesr_d)
            for i in range(3):
                nc.sync.dma_start(nws_sb[i], nws_d[i:i + 1, :])
            if use_be1:
                for i in range(3):
                    nc.sync.dma_start(wbe_sb[i], wbe_d[i:i + 1, :])
                nc.sync.dma_start(ones512_sb[:], ones512_d)

        def _late_const_dmas():
            nc.sync.dma_start(scale_sb[:], scalein_d)
            nc.sync.dma_start(bias_sb[:], biasin_d)
            nc.sync.dma_start(g2r_sb[:], g2r_d)
            for sb, d in ((g2_sb, g2_d), (be2_sb, be2_d)):
                nc.sync.dma_start(sb[:], d.rearrange("(j p) -> p j", p=P))
            nc.sync.dma_start(b1_sb[:], b1_d.rearrange("(j p) -> p j", p=P))
            if not use_be1:
                nc.sync.dma_start(ones512_sb[:], ones512_d)
            nc.sync.dma_start(b2s_sb[:], b2s_d)
            nc.sync.dma_start(masks_sb[:], masks_d.rearrange("o p t -> p o t"))

        def _sums(src_sl, sq_engines, ones, sq_dt):
            """s1/s2 column-sum matmuls; squares cycled over sq_engines.
            `ones`/`sq_dt` must match src dtype class (32-bit vs not)."""
            s1 = ps.tile([1, TB], F32, tag="acc", bufs=2, name="s1")
            s2 = ps.tile([1, TB], F32, tag="acc", bufs=2, name="s2")
            for j in range(NC):
                nc.tensor.matmul(s1[:], ones[:], src_sl[:, j, :],
                                 start=(j == 0), stop=(j == NC - 1))
            for j in range(NC):
                sq = work.tile([P, TB], sq_dt, tag="sq", bufs=2)
                eng = sq_engines[j % len(sq_engines)]
                if eng is nc.scalar:
                    nc.scalar.activation(sq[:], src_sl[:, j, :], AF.Square)
                else:
                    eng.tensor_mul(sq[:], src_sl[:, j, :], src_sl[:, j, :])
                nc.tensor.matmul(s2[:], ones[:], sq[:],
                                 start=(j == 0), stop=(j == NC - 1))
            return s1, s2

        def _muvar(s1, s2):
            mu = rows.tile([1, TB], F32R, tag="mu", bufs=2)
            t = rows.tile([1, TB], F32R, tag="tmp", bufs=2)
            r = rows.tile([1, TB], F32R, tag="r", bufs=2)
            nc.vector.tensor_scalar_mul(mu[:], s1[:], 1.0 / D)
            nc.vector.tensor_mul(t[:], mu[:], mu[:])
            nc.vector.scalar_tensor_tensor(t[:], s2[:], 1.0 / D, t[:],
                                           ALU.mult, ALU.subtract)
            nc.vector.tensor_scalar_add(t[:], t[:], EPS)
            nc.scalar.activation(t[:], t[:], AF.Sqrt)
            nc.vector.reciprocal(r[:], t[:])
            return mu, r, t

        def ln_stats(src_sl):
            """src_sl: [128, NC, TB] slice. Returns (r, mur) rows in SBUF.
            Avoids ACT entirely (it is exp-saturated during attention)."""
            s1, s2 = _sums(src_sl, (nc.vector, nc.gpsimd), onesc_sb, F32R)
            mu, r, _ = _muvar(s1, s2)
            mur = rows.tile([1, TB], F32R, tag="mur", bufs=1)
            nc.vector.tensor_mul(mur[:], mu[:], r[:])
            return r, mur

        def ln1_stats(tb, xt_t):
            """Stats for the folded LN1: returns (mu, bcrs, rcol).

            mu: [1,TB] f32r row (rank-1 correction rhs); bcrs: [P,TB] bf16
            broadcast of 1/sigma (columns); rcol: [P,4] f32 1/sigma along
            partitions for this block's four 128-token s-chunks (V scale).
            """
            sqe = (nc.scalar, nc.vector, nc.gpsimd) if tb == 0 \
                else (nc.scalar, nc.gpsimd)
            s1, s2 = _sums(xt_t[:], sqe, onescb_sb, BF16)
            mu, r, sig = _muvar(s1, s2)
            bcr = ps.tile([P, TB], F32, tag="yt", bufs=2, name="bcs")
            nc.tensor.matmul(bcr[:], onesr_sb[:], r[:], start=True, stop=True)
            bcrs = work.tile([P, TB], BF16, tag="bcrs")
            nc.vector.tensor_copy(bcrs[:], bcr[:])
            rtp = ps.tile([P, TB], F32, tag="yt", bufs=2, name="rtp")
            for si in range(4):
                nc.tensor.matmul(rtp[:, 4 * si:4 * si + 4],
                                 r[:, si * P:(si + 1) * P],
                                 onesr_sb[:, 0:4], start=True, stop=True)
            rcol = work.tile([P, 4], F32, tag="rcol")
            nc.vector.tensor_copy(
                rcol[:], rtp[:, 0:16].rearrange("p (a b) -> p a b",
                                                b=4)[:, :, 0])
            return mu, bcrs, rcol, sig

        def gen_ln_normalize(src_sl, dst_sl, r, mur, g_sb, be_sb, g_row):
            """dst = ((src*g[p])*bc(r) + be[p]) - bc(g[p]*mur).
            Generator: yields after each PE matmul batch. The r-broadcast is
            copied to SBUF at once so its PSUM slot frees immediately."""
            bcr = ps.tile([P, TB], F32, tag="yt", bufs=2, name="bcs")[:]
            nc.tensor.matmul(bcr, onesr_sb[:], r[:], start=True, stop=True)
            bcrw = work.tile([P, TB], BF16, tag="bcrs", name="bcrw")
            nc.vector.tensor_copy(bcrw[:], bcr)
            yield
            for j in range(NC):
                bc2 = ps.tile([P, TB], F32, tag="acc", bufs=2, name="bc2")
                nc.tensor.matmul(bc2[:], g_row[:, j * P:(j + 1) * P], mur[:],
                                 start=True, stop=True)
                t1 = work.tile([P, TB], F32R, tag="sq", bufs=2, name="t1")
                nc.vector.scalar_tensor_tensor(t1[:], src_sl[:, j, :],
                                               g_sb[:, j:j + 1], bcrw[:],
                                               ALU.mult, ALU.mult)
                nc.vector.scalar_tensor_tensor(dst_sl[:, j, :], t1[:],
                                               be_sb[:, j:j + 1], bc2[:],
                                               ALU.add, ALU.subtract)
                yield

        # ---------------- Phase 1+2: LN1-folded QKV, software-pipelined -----
        _mark(nc, "ln1")
        es_yt = ExitStack()
        p_yt = es_yt.enter_context(tc.tile_pool(name="p_yt", bufs=2,
                                                side="right"))
        yt0 = p_yt.tile([P, NC, TB], BF16, tag="yt_all", name="yt0")
        yt1 = p_yt.tile([P, NC, TB], BF16, tag="yt_all", name="yt1")

        es_kqv = ExitStack()
        p_kqv = es_kqv.enter_context(tc.tile_pool(name="p_kqv", bufs=1,
                                                  side="right"))
        kt_sb = p_kqv.tile([P, NC, T], BF16, tag="kt")      # K^T [m, s]
        qt_sb = p_kqv.tile([P, NC, 1024], BF16, tag="qt")   # Q^T [m, t_own]
        v_sb = p_kqv.tile([P, 16, H * 65], BF16, tag="v")   # V_ext [s, (h,65)]
        v_view = v_sb.rearrange("p s (h e) -> p s h e", e=65)
        nc.vector.memset(v_view[:, :, :, 64:65], 1.0)

        es_masks = ExitStack()
        p_masks = es_masks.enter_context(tc.tile_pool(name="p_masks", bufs=1,
                                                      side="right"))
        p_e = es_masks.enter_context(tc.tile_pool(name="p_e", bufs=4,
                                                  side="right"))
        masks_sb = p_masks.tile([P, 4, TB], BF16, tag="masks")
        # tri chunks (di>0) exp/mask/PV all operate only on the live column
        # sub-range [lo:], so the masked-out columns are never read and need
        # no pre-zeroed tiles.

        es_wqkv = ExitStack()
        p_wqkv = es_wqkv.enter_context(tc.tile_pool(name="p_wqkv", bufs=1,
                                                    side="right"))
        wq_sb = p_wqkv.tile([P, NC, D], BF16, tag="wq")
        wk_sb = p_wqkv.tile([P, NC, D], BF16, tag="wk")
        wv_sb = p_wqkv.tile([P, NC, D], BF16, tag="wv")

        def qkv_gen(tb, xt_t, mu, bcrs, rcol, sig):
            """QKV projections straight from raw x^T with LN1 folded in:
            psum accumulates (g1-folded w)@x plus the rank-1 -colsum(w)*mu
            correction; the per-token 1/sigma lands at psum readout (bcrs
            columns for K/Q, rcol per-partition scale for V).
            Accumulators live two-per [P, 2*TB] PSUM tile on the "st" tag
            (idle during phase 1). Generator: yields every ~3 matmuls so it
            can double as attention filler work."""
            tsl = slice(tb * TB, (tb + 1) * TB)
            _mark(nc, "qkv")

            def proj_pair(w_sb, ws_row, wbe_row, dst_sb, mtp):
                acc2 = ps.tile([P, 2 * TB], F32, tag="st", bufs=2, name="ka")
                for half in range(2):
                    mt = 2 * mtp + half
                    msl = slice(mt * P, (mt + 1) * P)
                    seg = acc2[:, half * TB:(half + 1) * TB]
                    for j in range(NC):
                        nc.tensor.matmul(seg, w_sb[:, j, msl], xt_t[:, j, :],
                                         start=(j == 0), stop=False)
                        if j == 2:
                            yield
                    nc.tensor.matmul(seg, ws_row[:, msl], mu[:],
                                     start=False, stop=(not use_be1))
                    if use_be1:
                        # be-term must survive the *1/sigma readout: rhs=sigma
                        nc.tensor.matmul(seg, wbe_row[:, msl], sig[:],
                                         start=False, stop=True)
                    yield
                for half in range(2):
                    mt = 2 * mtp + half
                    seg = acc2[:, half * TB:(half + 1) * TB]
                    nc.vector.tensor_mul(dst_sb[:, mt, tsl], seg, bcrs[:])

            for mtp in range(3):
                yield from proj_pair(wk_sb, nws_sb[1],
                                     wbe_sb[1] if use_be1 else None,
                                     kt_sb, mtp)
            for si in range(4):
                st = tb * 4 + si
                lsl = slice(si * P, (si + 1) * P)
                acc2 = ps.tile([P, 2 * TB], F32, tag="st", bufs=2, name="va")
                for half, fsl, off, w in ((0, slice(0, TB), 0, TB),
                                          (1, slice(TB, D), TB, D - TB)):
                    seg = acc2[:, off:off + w]
                    for j in range(NC):
                        nc.tensor.matmul(seg, xt_t[:, j, lsl],
                                         wv_sb[:, j, fsl],
                                         start=(j == 0), stop=False)
                        if j == 2:
                            yield
                    nc.tensor.matmul(seg, mu[:, lsl], nws_sb[2][:, fsl],
                                     start=False, stop=(not use_be1))
                    if use_be1:
                        nc.tensor.matmul(seg, sig[:, lsl], wbe_sb[2][:, fsl],
                                         start=False, stop=True)
                    yield
                for half, off, w in ((0, 0, TB), (1, TB, D - TB)):
                    src = acc2[:, off:off + w].rearrange(
                        "p (h e) -> p h e", e=64)
                    h0 = half * 8
                    nc.scalar.activation(
                        v_view[:, st, h0:h0 + w // 64, 0:64], src, AF.Copy,
                        scale=rcol[:, si:si + 1])
            if tb < 2:
                for mtp in range(3):
                    yield from proj_pair(wq_sb, nws_sb[0],
                                         wbe_sb[0] if use_be1 else None,
                                         qt_sb, mtp)

        def qkv_for_tb(*a):
            for _ in qkv_gen(*a):
                pass

        def attn_division(pend):
            # recip + broadcast + divide from the SBUF copies; deferred into
            # the NEXT mt's chunk stream so the bc matmuls never head-of-line
            # block the next mt's QKs
            yt_all, mt, yt_sbs = pend
            for ph in range(2):
                o = ph * 64
                yt_sb = yt_sbs[ph]
                rc = rows.tile([1, TB], F32R, tag="rc", bufs=2)
                nc.vector.reciprocal(rc[:], yt_sb[64:65, :])
                bc = ps.tile([64, TB], F32, tag="acc", bufs=2, name="abc")
                nc.tensor.matmul(bc[:], onesr_sb[:, 0:64],
                                 rc[:], start=True, stop=True)
                nc.vector.tensor_mul(yt_all[o:o + 64, mt, :],
                                     yt_sb[0:64, :], bc[:])

        def attn_mt(sl_i, yt_all, mt, fill, pend):
            chunks = SLOT_CHUNKS[sl_i]
            yt2 = [ps.tile([65, TB], F32, tag="yt", bufs=2,
                           name=f"yt_{sl_i}_{mt}_{ph}") for ph in range(2)]
            for ci, ch in enumerate(chunks):
                sb_idx = (0 if sl_i == 0 else 8) + ci
                di = ch - 4 * sl_i
                tri = 0 <= di < 4
                lo = 128 * di if tri else 0
                st2 = ps.tile([P, 2 * TB], F32, tag="st", bufs=2)
                qsub = slice(sl_i * TB + lo, (sl_i + 1) * TB)
                for ph in range(2):
                    o = ph * 64
                    nc.tensor.matmul(
                        st2[:, ph * TB + lo:(ph + 1) * TB],
                        kt_sb[o:o + 64, mt, ch * P:(ch + 1) * P],
                        qt_sb[o:o + 64, mt, qsub],
                        start=True, stop=True)
                e_tile = p_e.tile([P, 2 * TB], BF16, tag="e",
                                  name=f"e_{sl_i}_{mt}_{ci}")
                e_sb = e_tile[:]
                ev2 = e_sb.rearrange("p (two t) -> p two t", two=2)
                if lo:
                    ev = ev2[:, :, lo:]
                    sv = st2[:].rearrange("p (two t) -> p two t",
                                          two=2)[:, :, lo:]
                else:
                    ev, sv = e_sb, st2[:]
                nc.scalar.activation(
                    ev, sv, AF.Exp,
                    bias=bias_sb[:, sb_idx:sb_idx + 1],
                    scale=scale_sb[:, sb_idx:sb_idx + 1])
                if tri:
                    for ph in range(2):
                        nc.vector.tensor_mul(ev2[:, ph, lo:], ev2[:, ph, lo:],
                                             masks_sb[:, di, lo:])
                for ph in range(2):
                    h = 2 * mt + ph
                    nc.tensor.matmul(
                        yt2[ph][:, lo:], v_sb[:, ch, h * 65:(h + 1) * 65],
                        ev2[:, ph, lo:],
                        start=(ci == 0),
                        stop=(ci == len(chunks) - 1))
                if ci == 1 and pend is not None:
                    attn_division(pend)
                    pend = None
                if fill is not None:
                    next(fill, None)
            yt_sbs = []
            for ph in range(2):
                # copy [65,TB] to SBUF immediately: frees the PSUM bank so
                # the next mt's PV can start during the division
                yt_sb = work.tile([65, TB], F32R, tag="ydiv", bufs=3)
                nc.vector.tensor_copy(yt_sb[:], yt2[ph][:])
                yt_sbs.append(yt_sb)
            return (yt_all, mt, yt_sbs)


        with tc.tile_pool(name="p_xtr", bufs=3) as p_xtr:
            stats = {}
            xts = {}
            for tb in range(NTB):
                tsl = slice(tb * TB, (tb + 1) * TB)
                xt_t = p_xtr.tile([P, NC, TB], BF16, tag="xtr")
                if tb == 0:
                    # tiny consts first (s1's onesc lhsT must not queue
                    # behind the bulk transfers), then the first block split
                    # so the stats tree can start after ~1/3 of it
                    _early_const_dmas()
                    for jj in range(3):
                        nc.sync.dma_start(xt_t[:, 2 * jj:2 * jj + 2, :],
                                          xtb_r[:, 2 * jj:2 * jj + 2, tsl])
                else:
                    nc.sync.dma_start(xt_t[:], xtb_r[:, :, tsl])
                if tb == 0:
                    nc.sync.dma_start(wk_sb[:],
                                      wkt_d.rearrange("(j p) m -> p j m", p=P))
                    nc.sync.dma_start(wv_sb[:],
                                      wvt_d.rearrange("(j p) m -> p j m", p=P))
                    nc.sync.dma_start(wq_sb[:],
                                      wqt_d.rearrange("(j p) m -> p j m", p=P))
                if tb == 1:
                    _late_const_dmas()
                xts[tb] = xt_t
                # emit qkv(tb-1) before stats(tb): the bulk PE work is ready
                # to run, so the in-order PE stream never parks on the
                # square/sum chain of the next block
                if tb > 0:
                    qkv_for_tb(tb - 1, xts[tb - 1][:], *stats[tb - 1])
                stats[tb] = ln1_stats(tb, xt_t)
            # slot0's first two mt's only need tb0/tb2 projections: run them
            # with qkv(3) threaded through as filler, so their exps use
            # phase-1's idle ACT while the last projection block feeds PE
            _mark(nc, "attn")
            gq = qkv_gen(NTB - 1, xts[NTB - 1][:], *stats[NTB - 1])
            pend0 = attn_mt(0, yt0, 0, gq, None)
            pend0 = attn_mt(0, yt0, 1, gq, pend0)
            for _ in gq:
                pass
        es_wqkv.close()

        # ---------------- Phase 3: attention (+ per-slot wo/LN2) -----------
        # Slot 1's chunk loop is exp(ACT)-bound with ~190ns/chunk of PE
        # slack, so slot 0's w_o projection, LN2 and the first MLP1 groups
        # are threaded through it as fine-grained fillers (one small PE
        # batch per attention chunk).
        _mark(nc, "attn")
        w1t_r = w1t_d.rearrange("g (j p) two f -> p g j two f", p=P)
        w2t_r = w2t_d.rearrange("(f p) two c -> p f two c", p=P)
        outt_r = outt_d.rearrange("(j p) t -> p j t", p=P)
        p_xp = ctx.enter_context(tc.tile_pool(name="p_xp", bufs=1))
        xp_sb = p_xp.tile([P, NC, 1024], F32R, tag="xp")
        p_xn2 = ctx.enter_context(tc.tile_pool(name="p_xn2", bufs=1))
        xn2_sb = p_xn2.tile([P, NC, 1024], F8, tag="xn2")
        p_h1a = ctx.enter_context(tc.tile_pool(name="p_h1a", bufs=1))
        p_wmlp = ctx.enter_context(tc.tile_pool(name="p_wmlp", bufs=2))
        h1s = {0: p_h1a.tile([P, NF, TB], F8, tag="h1a", name="h1a"),
               1: None}
        es_wo = ExitStack()
        p_wo = es_wo.enter_context(tc.tile_pool(name="p_wo", bufs=1))
        wo_sb = p_wo.tile([P, NC, D], BF16, tag="wo")
        nc.sync.dma_start(wo_sb[:], wo_d.rearrange("(j p) m -> p j m", p=P))
        xo0_sb = p_wo.tile([P, NC, TB], BF16, tag="xo", bufs=1, name="xo0")
        nc.sync.dma_start(xo0_sb[:], xtb_r[:, :, 0:TB])
        xo_hold = {0: xo0_sb}

        def gen_wo(sl_i, yt_all):
            _mark(nc, "wo")
            qsl = slice(sl_i * TB, (sl_i + 1) * TB)
            xo_t = xo_hold[sl_i]
            for ct in range(NC):
                ao = ps.tile([P, TB], F32, tag="acc", bufs=2, name="ao")
                for mc in range(NC):
                    nc.tensor.matmul(ao[:],
                                     wo_sb[:, mc, ct * P:(ct + 1) * P],
                                     yt_all[:, mc, :],
                                     start=(mc == 0), stop=(mc == NC - 1))
                    if mc % 2:
                        yield
                nc.vector.tensor_add(xp_sb[:, ct, qsl], xo_t[:, ct, :],
                                     ao[:])

        def gen_xo1():
            # slot1 residual load: reuses xo0's slot, so it must be emitted
            # only after all of gen_wo(0)'s reads
            xo1 = p_wo.tile([P, NC, TB], BF16, tag="xo", bufs=1, name="xo1")
            nc.sync.dma_start(xo1[:], xtb_r[:, :, TB:2 * TB])
            xo_hold[1] = xo1
            return
            yield

        def gen_ln2(sl_i):
            _mark(nc, "ln2")
            qsl = slice(sl_i * TB, (sl_i + 1) * TB)
            src = xp_sb[:, :, qsl]
            s1 = ps.tile([1, TB], F32, tag="acc", bufs=2, name="s1")
            s2 = ps.tile([1, TB], F32, tag="acc", bufs=2, name="s2")
            for j in range(NC):
                nc.tensor.matmul(s1[:], onesc_sb[:], src[:, j, :],
                                 start=(j == 0), stop=(j == NC - 1))
                if j % 2:
                    yield
            sqe = (nc.vector, nc.gpsimd)
            for j in range(NC):
                sq = work.tile([P, TB], F32R, tag="sq", bufs=2)
                sqe[j % 2].tensor_mul(sq[:], src[:, j, :], src[:, j, :])
                nc.tensor.matmul(s2[:], onesc_sb[:], sq[:],
                                 start=(j == 0), stop=(j == NC - 1))
                if j % 2:
                    yield
            mu2, r2, _ = _muvar(s1, s2)
            mur2 = rows.tile([1, TB], F32R, tag="mur", bufs=1)
            nc.vector.tensor_mul(mur2[:], mu2[:], r2[:])
            yield
            yield from gen_ln_normalize(src, xn2_sb[:, :, qsl], r2, mur2,
                                        g2_sb, be2_sb, g2r_sb)

        def gen_mlp1(groups, tbs, after_group=None):
            _mark(nc, "mlp1")
            for ft4 in groups:
                w1_t = p_wmlp.tile([P, NC, 2, 4 * P], F8, tag="w1", bufs=2)
                nc.sync.dma_start(w1_t[:], w1t_r[:, ft4])
                for sub in range(4):
                    ft = 4 * ft4 + sub
                    for tb in tbs:
                        tsl = slice(tb * TB, (tb + 1) * TB)
                        hp = ps.tile([P, TB], F32, tag="acc", bufs=2,
                                     name="hp")
                        for j in range(NC):
                            nc.tensor.matmul(
                                hp[:],
                                w1_t[:, j, :, sub * P:(sub + 1) * P],
                                xn2_sb[:, j, tsl].unsqueeze(1)
                                .broadcast_to([P, 2, TB]),
                                start=(j == 0), stop=(j == NC - 1),
                                perf_mode=DR)
                            if j % 3 == 2:
                                yield
                        # h1 = relu(psum + S_H1*b1) (S_H1 == S_W1 so scale=1)
                        if use_b1:
                            nc.scalar.activation(h1s[tb][:, ft, :], hp[:],
                                                 AF.Relu,
                                                 bias=b1_sb[:, ft:ft + 1],
                                                 scale=S_H1 / S_W1)
                        else:
                            nc.vector.tensor_scalar_max(h1s[tb][:, ft, :],
                                                        hp[:], 0.0)
                        yield
                if after_group is not None:
                    after_group(ft4)

        # slot 0 rest (mts 0-1 already ran inside phase 1)
        for mt in range(2, NC):
            pend0 = attn_mt(0, yt0, mt, None, pend0)
        # slot 1 with interleaved fillers
        import itertools as _it
        fill = _it.chain(gen_wo(0, yt0), gen_xo1(), gen_ln2(0),
                         gen_mlp1(range(0, 3), (0,)))
        for mt in range(NC):
            pend0 = attn_mt(1, yt1, mt, fill, pend0)
        attn_division(pend0)
        for _ in fill:
            pass

        es_masks.close()
        es_kqv.close()

        # prefetch the first tail-MLP1 w1 groups so their transfers run
        # under wo(slot1)/LN2(slot1)
        tail_items = [(3, 0), (4, 0), (5, 0)] + [(g, 1) for g in range(6)]
        w1_tiles = {}

        def _issue_w1(i):
            if i < len(tail_items):
                t = p_wmlp.tile([P, NC, 2, 4 * P], F8, tag="w1", bufs=2,
                                name=f"w1b_{i}")
                nc.sync.dma_start(t[:], w1t_r[:, tail_items[i][0]])
                w1_tiles[i] = t

        for i in range(2):
            _issue_w1(i)
        for _ in gen_wo(1, yt1):
            pass
        for _ in gen_ln2(1):
            pass

        es_wo.close()
        es_yt.close()

        # ---------------- Phase 6: MLP tail ----------------
        _mark(nc, "mlp")
        with tc.tile_pool(name="p_h1b", bufs=1) as p_h1b, \
             tc.tile_pool(name="p_w2", bufs=1) as p_w2, \
             tc.tile_pool(name="p_out", bufs=4) as p_out:
            h1s[1] = p_h1b.tile([P, NF, TB], F8, tag="h1b", name="h1b")
            w2_sb = p_w2.tile([P, NF, 2, D], F8, tag="w2full")
            # rest of MLP1 as a depth-3 w1-prefetch pipeline; w2 streams in
            # behind the w1 loads
            for i, (ft4, tb) in enumerate(tail_items):
                w1_t = w1_tiles.pop(i)
                tsl = slice(tb * TB, (tb + 1) * TB)
                for sub in range(4):
                    ft = 4 * ft4 + sub
                    hp = ps.tile([P, TB], F32, tag="acc", bufs=2, name="hp")
                    for j in range(NC):
                        nc.tensor.matmul(
                            hp[:], w1_t[:, j, :, sub * P:(sub + 1) * P],
                            xn2_sb[:, j, tsl].unsqueeze(1)
                            .broadcast_to([P, 2, TB]),
                            start=(j == 0), stop=(j == NC - 1),
                            perf_mode=DR)
                    nc.scalar.activation(h1s[tb][:, ft, :], hp[:], AF.Relu,
                                         bias=b1_sb[:, ft:ft + 1],
                                         scale=S_H1 / S_W1)
                _issue_w1(i + 2)
                if i < 4:
                    nc.sync.dma_start(w2_sb[:, 6 * i:6 * i + 6],
                                      w2t_r[:, 6 * i:6 * i + 6, :, :])
            # MLP2: uneven 4+2 column grouping so the last-finishing group
            # has only 4 output DMAs in the tail
            for cts in (range(0, 4), range(4, 6)):
                cts = list(cts)
                o2s = {}
                for idx, ct in enumerate(cts):
                    if idx < 2:
                        tag = ("acc", "yt")[idx]
                        for tb in range(2):
                            o2s[(ct, tb)] = ps.tile(
                                [P, TB], F32, tag=tag, bufs=2,
                                name=f"o2_{ct}_{tb}")
                    else:
                        stp = ps.tile([P, 2 * TB], F32, tag="st", bufs=2,
                                      name=f"o2st_{ct}")
                        o2s[(ct, 0)] = stp[:, 0:TB]
                        o2s[(ct, 1)] = stp[:, TB:2 * TB]
                # seed each accumulator with (b2/C_MLP) x ones
                for ct in cts:
                    for tb in range(2):
                        nc.tensor.matmul(o2s[(ct, tb)][:],
                                         b2s_sb[:, ct * P:(ct + 1) * P],
                                         ones512_sb[:],
                                         start=True, stop=False)
                for ft in range(NF):
                    for tb in range(2):
                        rhs = h1s[tb][:, ft, :].unsqueeze(1).broadcast_to(
                            [P, 2, TB])
                        for ct in cts:
                            nc.tensor.matmul(
                                o2s[(ct, tb)][:],
                                w2_sb[:, ft, :, ct * P:(ct + 1) * P],
                                rhs, start=False, stop=(ft == NF - 1),
                                perf_mode=DR)
                for tb in range(2):
                    tsl = slice(tb * TB, (tb + 1) * TB)
                    for ct in cts:
                        ot = p_out.tile([P, TB], F32, tag="ot",
                                        name=f"ot_{ct}_{tb}")
                        nc.vector.scalar_tensor_tensor(
                            ot[:], o2s[(ct, tb)][:], C_MLP,
                            xp_sb[:, ct, tsl], ALU.mult, ALU.add)
                        nc.sync.dma_start(outt_r[:, ct, tsl], ot[:])

    nc.compile()
    return nc


def _hilo(w, f8):
    """[..., n] -> [..., 2, n] fp8 (hi, residual-lo) planes."""
    hi = w.astype(f8)
    lo = (w - hi.astype(np.float32)).astype(f8)
    return np.ascontiguousarray(np.stack([hi, lo], axis=-2))


def _host_inputs(X, w_q, w_k, w_v, w_o, W1, b1, W2, b2, g1, be1, g2, be2):
    """Build the 8 per-core input dicts."""
    f32 = np.float32
    import ml_dtypes as _mld
    _f8 = _mld.float8_e4m3
    _bf = _mld.bfloat16
    g1v = np.asarray(g1, f32)
    be1v = np.asarray(be1, f32)
    # LN1 fold: g1 into the QKV weight columns; mean correction rows are the
    # negated column sums; optional be1 rows handle a nonzero LN1 shift
    wqg = np.asarray(w_q, f32).reshape(D, D) * g1v[None, :]
    wkg = np.asarray(w_k, f32).reshape(D, D) * g1v[None, :]
    wvg = np.asarray(w_v, f32).reshape(D, D) * g1v[None, :]
    wqt = np.ascontiguousarray(wqg.T.astype(_bf))
    wkt = np.ascontiguousarray(wkg.T.astype(_bf))
    wvt = np.ascontiguousarray(wvg.T.astype(_bf))
    nws = np.ascontiguousarray(np.stack(
        [-wqg.sum(axis=1), -wkg.sum(axis=1), -wvg.sum(axis=1)]).astype(f32))
    use_be1 = bool(np.any(be1v))
    wo = np.ascontiguousarray(np.asarray(w_o, f32).astype(_bf))
    w1t = None  # bf16, set below
    w2t = None  # bf16, set below
    onesr = np.ones((1, P), f32)
    onesc = np.ones((P, 1), f32)
    onescb = np.ones((P, 1), _bf)
    onesv = None  # set below after bf16 import
    # 4 canonical self-diagonal masks: mask[k][s, t] = (128k + s <= t)
    import ml_dtypes
    bf16 = ml_dtypes.bfloat16
    masks = np.zeros((4, P, TB), bf16)
    ar_s = np.arange(P)[:, None]
    ar_t = np.arange(TB)[None, :]
    for k in range(4):
        masks[k] = (128 * k + ar_s <= ar_t).astype(bf16)
    w1t = _hilo(np.asarray(W1, f32).T * S_W1, _f8)   # [D, 2, DFF]
    w1t = np.ascontiguousarray(
        w1t.reshape(D, 2, NF // 4, 4 * P).transpose(2, 0, 1, 3))
    w2t = _hilo(np.asarray(W2, f32).T * S_W2, _f8)

    # per-role exp scale/bias: 24 = 8 (slot0) + 16 (slot1) chunk positions
    sc = {}
    bi = {}
    for role in range(2):
        order = ROLE_ORDER[role]
        s = np.full((24,), 0.125, f32)
        b = np.zeros((24,), f32)
        for sl_i in range(2):
            own_blk = order[sl_i]
            for ci, ch in enumerate(SLOT_CHUNKS[sl_i]):
                idx = (0 if sl_i == 0 else 8) + ci
                pos = ch // 4           # permuted 512-block of this s-chunk
                blk = order[pos]
                if pos == sl_i or blk < own_blk:
                    pass                # diagonal (tri-masked) or past: live
                else:
                    s[idx] = 0.0        # future: dead
                    b[idx] = DEAD
        sc[role] = np.broadcast_to(s, (P, 24)).copy()
        bi[role] = np.broadcast_to(b, (P, 24)).copy()

    g2r = np.asarray(g2, f32).reshape(1, D)
    shared = dict(wqt=wqt, wkt=wkt, wvt=wvt, wo=wo, w1t=w1t, w2t=w2t,
                  g2r=g2r, nws=nws,
                  onesr=onesr, onesc=onesc, onescb=onescb, masks=masks,
                  g2v=np.asarray(g2, f32), be2v=np.asarray(be2, f32),
                  b1v=np.asarray(b1, f32) * S_H1,
                  b2s=np.asarray(b2, f32).reshape(1, D) * (S_H1 * S_W2),
                  ones512=np.ones((1, TB), f32))
    if use_be1:
        shared["wbe"] = np.ascontiguousarray(np.stack(
            [np.asarray(w, f32).reshape(D, D) @ be1v
             for w in (w_q, w_k, w_v)]).astype(f32))

    in_maps = []
    for core in range(8):
        role, b_idx = core // 4, core % 4
        order = ROLE_ORDER[role]
        xb = np.asarray(X[b_idx], f32)          # [T, D]
        xperm = np.concatenate([xb[o * TB:(o + 1) * TB] for o in order], axis=0)
        xt = np.ascontiguousarray(xperm.T)      # [D, T]
        m = dict(shared)
        m["xt"] = xt
        m["xtb"] = np.ascontiguousarray(xt.astype(_bf))
        m["scalein"] = sc[role]
        m["biasin"] = bi[role]
        in_maps.append(m)
    return in_maps


def _assemble(results, dtype):
    out = np.empty((B, T, D), dtype)
    for core in range(8):
        role, b_idx = core // 4, core % 4
        order = ROLE_ORDER[role]
        ot = results[core]["outt"]              # [D, 1024]
        for sl_i in range(2):
            blk = order[sl_i]
            out[b_idx, blk * TB:(blk + 1) * TB] = \
                ot[:, sl_i * TB:(sl_i + 1) * TB].T
    return out


def kernel(X, w_q, w_k, w_v, w_o, W1, b1, W2, b2, g1, be1, g2, be2,
           _want_results=False, _trace=False):
    use_be1 = bool(np.any(np.asarray(be1)))
    use_b1 = bool(np.any(np.asarray(b1)))
    key = ("nc", use_be1, use_b1)
    if key not in _cached:
        _cached[key] = _build_nc(use_be1=use_be1, use_b1=use_b1)
        _cached["nc"] = _cached[key]
    nc = _cached[key]
    in_maps = _host_inputs(X, w_q, w_k, w_v, w_o, W1, b1, W2, b2,
                           g1, be1, g2, be2)
    res = run_bass_kernel_spmd(nc, in_maps, core_ids=list(range(8)),
                               trace=_trace)
    out = _assemble(res.results, np.asarray(X).dtype)
    if _want_results:
        return out, res
    return out



# revision 96
# speedup vs baseline: 1.0105x; 1.0044x over previous
"""Trainium2 Bass kernel for a dense transformer block (pre-LN attn + MLP).

B=4, T=2048, D=768, H=12 (DH=64), DFF=3072, fp32.

Sharding: 8 cores = 4 batches x 2 roles. Each core processes one batch and
owns 1024 query tokens (two 512-blocks, paired {0,3}/{1,2} for causal load
balance). K/V are computed for the full 2048 tokens on both cores of a batch
(cheap), so there are NO collectives.

SPMD uniformity: all 8 cores run ONE identical NEFF. Causal structure is
carried in DATA, not code:
  - host permutes each batch's token axis to [own0, own1, otherA, otherB]
  - q-slot0 attends s-chunks {0..3, 8..11}; q-slot1 attends s-chunks {0..15}
  - per-(slot,chunk) exp scale/bias inputs select live / dead (zero) chunks
  - 4 canonical triangular masks handle the self-diagonal 512-blocks

Everything on-chip runs in a transposed layout (features on partitions,
tokens on the free axis) so no on-chip transposes are needed; all weight /
input transposes happen on the host in numpy. Matmuls run as float32r
(full PE speed, ~bf16x2 precision). LayerNorm statistics are computed with
ones-column matmuls; per-token stats are broadcast across partitions with
K=1 outer-product matmuls. Softmax denominators come for free from a ones
column appended to V (65-row PV matmul); the divide is folded in after PV.
"""

import sys

sys.path.insert(0, "/opt/trn_rl_repo")

from contextlib import ExitStack

import numpy as np

import concourse.bass as bass
import concourse.mybir as mybir
import concourse.tile as tile
from concourse import bacc
from concourse.bass_utils import run_bass_kernel_spmd

F32 = mybir.dt.float32
F32R = mybir.dt.float32r
AF = mybir.ActivationFunctionType
BF16 = mybir.dt.bfloat16
F8 = mybir.dt.float8e4
DR = mybir.MatmulPerfMode.DoubleRow
ALU = mybir.AluOpType

# fp8 pre-scales (host folds these into the weights; kernel divides out)
S_W1 = 16.0     # W1 stored as 16*W1
S_H1 = 16.0     # h1 stored as 16*relu(.) == relu(psum) directly (scale 1)
S_W2 = 16.0     # W2 stored as 16*W2
C_MLP = 1.0 / (S_H1 * S_W2)          # o2 -> true h@W2.T scale
S_YDIV = 1.0    # PV sums pre-scale before the divide (1 = none)
C_WO = 1.0      # wo-out descale folded into the residual add

H, D, DFF = 12, 768, 3072
DH = 64
B, T = 4, 2048
EPS = 1e-5
P = 128
NC = D // P          # 6 feature chunks
NF = DFF // P        # 24 ff tiles
TB = 512             # token block
NTB = T // TB        # 4 blocks
SLOT_CHUNKS = [[0, 1, 2, 3, 8, 9, 10, 11], list(range(16))]
# role -> permuted block order [own0, own1, restA, restB] (original block ids)
ROLE_ORDER = [[0, 3, 1, 2], [1, 2, 0, 3]]
DEAD = -30000.0      # exp(DEAD) == 0 in fp32

_cached = {}
PHASE_MARKS = []


def _mark(nc, name):
    PHASE_MARKS.append((name, nc.next_id()))


def _build_nc(use_be1=False, use_b1=False):
    nc = bacc.Bacc("TRN2", target_bir_lowering=False, debug=False,
                   enable_asserts=False, num_devices=8)

    def din(name, shape, dt=F32R):
        return nc.dram_tensor(name, shape, dt, kind="ExternalInput").ap()

    xt_d = din("xt", [D, T])                 # X[b].T, token-permuted (f32)
    xtb_d = din("xtb", [D, T], BF16)         # same, bf16 (QKV matmul input)
    wqt_d = din("wqt", [D, D], BF16)         # g1-folded w_q as [c, m]
    wkt_d = din("wkt", [D, D], BF16)
    wvt_d = din("wvt", [D, D], BF16)
    wo_d = din("wo", [D, D], BF16)           # natural [m, c]
    w1t_d = din("w1t", [NF // 4, D, 2, 4 * P], F8)  # [fgrp, c, (hi/lo), f]
    w2t_d = din("w2t", [DFF, 2, D], F8)      # 16*W2.T  [f, (hi/lo), c]
    ones512_d = din("ones512", [1, TB])      # ones row for b2 outer
    b2s_d = din("b2s", [1, D])               # 128*b2 as a row
    onesr_d = din("onesr", [1, P])           # outer-product lhsT
    onesc_d = din("onesc", [P, 1])           # column-sum lhsT (f32)
    onescb_d = din("onescb", [P, 1], BF16)   # column-sum lhsT (bf16)
    masks_d = din("masks", [4, P, TB], BF16)  # tri masks (shared by ph)
    scalein_d = din("scalein", [P, 24], F32) # exp scale per (slot,chunk)
    biasin_d = din("biasin", [P, 24], F32)   # exp bias per (slot,chunk)
    # LN1 is folded into the QKV path: -colsum(g1-folded w) rows (q, k, v)
    # for the rank-1 mean correction (and optional be1-fold rows)
    nws_d = din("nws", [3, D])
    if use_be1:
        wbe_d = din("wbe", [3, D])
    g2_d = din("g2v", [D], F32)
    be2_d = din("be2v", [D], F32)
    g2r_d = din("g2r", [1, D])
    b1_d = din("b1v", [DFF], F32)

    outt_d = nc.dram_tensor("outt", [D, 1024], F32, kind="ExternalOutput").ap()

    xt_r = xt_d.rearrange("(j p) t -> p j t", p=P)
    xtb_r = xtb_d.rearrange("(j p) t -> p j t", p=P)

    with tile.TileContext(nc) as tc, ExitStack() as ctx, \
         nc.allow_low_precision(reason="fp32r/bf16 intermediates are intended"):
        consts = ctx.enter_context(tc.tile_pool(name="consts", bufs=1))
        ps = ctx.enter_context(tc.tile_pool(name="ps", bufs=1, space="PSUM"))
        rows = ctx.enter_context(tc.tile_pool(name="rows", bufs=1))
        work = ctx.enter_context(tc.tile_pool(name="work", bufs=2))

        onesr_sb = consts.tile([1, P], F32R, tag="onesr")
        onesc_sb = consts.tile([P, 1], F32R, tag="onesc")
        onescb_sb = consts.tile([P, 1], BF16, tag="onescb")
        scale_sb = consts.tile([P, 24], F32, tag="scalein")
        bias_sb = consts.tile([P, 24], F32, tag="biasin")
        g2_sb = consts.tile([P, NC], F32, tag="g2")
        be2_sb = consts.tile([P, NC], F32, tag="be2")
        g2r_sb = consts.tile([1, D], F32R, tag="g2r")
        b1_sb = consts.tile([P, NF], F32, tag="b1")
        ones512_sb = consts.tile([1, TB], F32R, tag="ones512")
        b2s_sb = consts.tile([1, D], F32R, tag="b2s")
        # q/k/v correction rows (matmul operands must share base partition 0)
        nws_sb = {i: consts.tile([1, D], F32R, tag=f"nws{i}", name=f"nws{i}")
                  for i in range(3)}
        if use_be1:
            wbe_sb = {i: consts.tile([1, D], F32R, tag=f"wbe{i}",
                                     name=f"wbe{i}") for i in range(3)}

        def _early_const_dmas():
            nc.sync.dma_start(onesc_sb[:], onesc_d)
            nc.sync.dma_start(onesr_sb[:], onesr_d)
            for i in range(3):
                nc.sync.dma_start(nws_sb[i], nws_d[i:i + 1, :])
            if use_be1:
                for i in range(3):
                    nc.sync.dma_start(wbe_sb[i], wbe_d[i:i + 1, :])
                nc.sync.dma_start(ones512_sb[:], ones512_d)

        def _late_const_dmas():
            nc.sync.dma_start(scale_sb[:], scalein_d)
            nc.sync.dma_start(bias_sb[:], biasin_d)
            nc.sync.dma_start(g2r_sb[:], g2r_d)
            for sb, d in ((g2_sb, g2_d), (be2_sb, be2_d)):
                nc.sync.dma_start(sb[:], d.rearrange("(j p) -> p j", p=P))
            nc.sync.dma_start(b1_sb[:], b1_d.rearrange("(j p) -> p j", p=P))
            if not use_be1:
                nc.sync.dma_start(ones512_sb[:], ones512_d)
            nc.sync.dma_start(b2s_sb[:], b2s_d)
            nc.sync.dma_start(masks_sb[:], masks_d.rearrange("o p t -> p o t"))

        def _sums(src_sl, sq_engines, ones, sq_dt):
            """s1/s2 column-sum matmuls; squares cycled over sq_engines.
            `ones`/`sq_dt` must match src dtype class (32-bit vs not)."""
            s1 = ps.tile([1, TB], F32, tag="acc", bufs=2, name="s1")
            s2 = ps.tile([1, TB], F32, tag="acc", bufs=2, name="s2")
            for j in range(NC):
                nc.tensor.matmul(s1[:], ones[:], src_sl[:, j, :],
                                 start=(j == 0), stop=(j == NC - 1))
            for j in range(NC):
                sq = work.tile([P, TB], sq_dt, tag="sq", bufs=2)
                eng = sq_engines[j % len(sq_engines)]
                if eng is nc.scalar:
                    nc.scalar.activation(sq[:], src_sl[:, j, :], AF.Square)
                else:
                    eng.tensor_mul(sq[:], src_sl[:, j, :], src_sl[:, j, :])
                nc.tensor.matmul(s2[:], ones[:], sq[:],
                                 start=(j == 0), stop=(j == NC - 1))
            return s1, s2

        def _muvar(s1, s2):
            mu = rows.tile([1, TB], F32R, tag="mu", bufs=2)
            t = rows.tile([1, TB], F32R, tag="tmp", bufs=2)
            r = rows.tile([1, TB], F32R, tag="r", bufs=2)
            nc.vector.tensor_scalar_mul(mu[:], s1[:], 1.0 / D)
            nc.vector.tensor_mul(t[:], mu[:], mu[:])
            nc.vector.scalar_tensor_tensor(t[:], s2[:], 1.0 / D, t[:],
                                           ALU.mult, ALU.subtract)
            nc.vector.tensor_scalar_add(t[:], t[:], EPS)
            nc.scalar.activation(t[:], t[:], AF.Sqrt)
            nc.vector.reciprocal(r[:], t[:])
            return mu, r, t

        def ln_stats(src_sl):
            """src_sl: [128, NC, TB] slice. Returns (r, mur) rows in SBUF.
            Avoids ACT entirely (it is exp-saturated during attention)."""
            s1, s2 = _sums(src_sl, (nc.vector, nc.gpsimd), onesc_sb, F32R)
            mu, r, _ = _muvar(s1, s2)
            mur = rows.tile([1, TB], F32R, tag="mur", bufs=1)
            nc.vector.tensor_mul(mur[:], mu[:], r[:])
            return r, mur

        def ln1_stats(tb, xt_t):
            """Stats for the folded LN1: returns (mu, bcrs, rcol).

            mu: [1,TB] f32r row (rank-1 correction rhs); bcrs: [P,TB] bf16
            broadcast of 1/sigma (columns); rcol: [P,4] f32 1/sigma along
            partitions for this block's four 128-token s-chunks (V scale).
            """
            sqe = (nc.scalar, nc.vector, nc.gpsimd) if tb == 0 \
                else (nc.scalar, nc.gpsimd)
            s1, s2 = _sums(xt_t[:], sqe, onescb_sb, BF16)
            mu, r, sig = _muvar(s1, s2)
            bcr = ps.tile([P, TB], F32, tag="yt", bufs=2, name="bcs")
            nc.tensor.matmul(bcr[:], onesr_sb[:], r[:], start=True, stop=True)
            bcrs = work.tile([P, TB], BF16, tag="bcrs")
            nc.vector.tensor_copy(bcrs[:], bcr[:])
            rtp = ps.tile([P, TB], F32, tag="yt", bufs=2, name="rtp")
            for si in range(4):
                nc.tensor.matmul(rtp[:, 4 * si:4 * si + 4],
                                 r[:, si * P:(si + 1) * P],
                                 onesr_sb[:, 0:4], start=True, stop=True)
            rcol = work.tile([P, 4], F32, tag="rcol")
            nc.vector.tensor_copy(
                rcol[:], rtp[:, 0:16].rearrange("p (a b) -> p a b",
                                                b=4)[:, :, 0])
            return mu, bcrs, rcol, sig

        def gen_ln_normalize(src_sl, dst_sl, r, mur, g_sb, be_sb, g_row):
            """dst = ((src*g[p])*bc(r) + be[p]) - bc(g[p]*mur).
            Generator: yields after each PE matmul batch. The r-broadcast is
            copied to SBUF at once so its PSUM slot frees immediately."""
            bcr = ps.tile([P, TB], F32, tag="yt", bufs=2, name="bcs")[:]
            nc.tensor.matmul(bcr, onesr_sb[:], r[:], start=True, stop=True)
            bcrw = work.tile([P, TB], BF16, tag="bcrs", name="bcrw")
            nc.vector.tensor_copy(bcrw[:], bcr)
            yield
            for j in range(NC):
                bc2 = ps.tile([P, TB], F32, tag="acc", bufs=2, name="bc2")
                nc.tensor.matmul(bc2[:], g_row[:, j * P:(j + 1) * P], mur[:],
                                 start=True, stop=True)
                t1 = work.tile([P, TB], F32R, tag="sq", bufs=2, name="t1")
                nc.vector.scalar_tensor_tensor(t1[:], src_sl[:, j, :],
                                               g_sb[:, j:j + 1], bcrw[:],
                                               ALU.mult, ALU.mult)
                nc.vector.scalar_tensor_tensor(dst_sl[:, j, :], t1[:],
                                               be_sb[:, j:j + 1], bc2[:],
                                               ALU.add, ALU.subtract)
                yield

        # ---------------- Phase 1+2: LN1-folded QKV, software-pipelined -----
        _mark(nc, "ln1")
        es_yt = ExitStack()
        p_yt = es_yt.enter_context(tc.tile_pool(name="p_yt", bufs=2,
                                                side="right"))
        yt0 = p_yt.tile([P, NC, TB], BF16, tag="yt_all", name="yt0")
        yt1 = p_yt.tile([P, NC, TB], BF16, tag="yt_all", name="yt1")

        es_kqv = ExitStack()
        p_kqv = es_kqv.enter_context(tc.tile_pool(name="p_kqv", bufs=1,
                                                  side="right"))
        kt_sb = p_kqv.tile([P, NC, T], BF16, tag="kt")      # K^T [m, s]
        qt_sb = p_kqv.tile([P, NC, 1024], BF16, tag="qt")   # Q^T [m, t_own]
        v_sb = p_kqv.tile([P, 16, H * 65], BF16, tag="v")   # V_ext [s, (h,65)]
        v_view = v_sb.rearrange("p s (h e) -> p s h e", e=65)
        nc.vector.memset(v_view[:, :, :, 64:65], 1.0)

        es_masks = ExitStack()
        p_masks = es_masks.enter_context(tc.tile_pool(name="p_masks", bufs=1,
                                                      side="right"))
        p_e = es_masks.enter_context(tc.tile_pool(name="p_e", bufs=4,
                                                  side="right"))
        masks_sb = p_masks.tile([P, 4, TB], BF16, tag="masks")
        # tri chunks (di>0) exp/mask/PV all operate only on the live column
        # sub-range [lo:], so the masked-out columns are never read and need
        # no pre-zeroed tiles.

        es_wqkv = ExitStack()
        p_wqkv = es_wqkv.enter_context(tc.tile_pool(name="p_wqkv", bufs=1,
                                                    side="right"))
        wq_sb = p_wqkv.tile([P, NC, D], BF16, tag="wq")
        wk_sb = p_wqkv.tile([P, NC, D], BF16, tag="wk")
        wv_sb = p_wqkv.tile([P, NC, D], BF16, tag="wv")

        def qkv_gen(tb, xt_t, mu, bcrs, rcol, sig):
            """QKV projections straight from raw x^T with LN1 folded in:
            psum accumulates (g1-folded w)@x plus the rank-1 -colsum(w)*mu
            correction; the per-token 1/sigma lands at psum readout (bcrs
            columns for K/Q, rcol per-partition scale for V).
            Accumulators live two-per [P, 2*TB] PSUM tile on the "st" tag
            (idle during phase 1). Generator: yields every ~3 matmuls so it
            can double as attention filler work."""
            tsl = slice(tb * TB, (tb + 1) * TB)
            _mark(nc, "qkv")

            def proj_pair(w_sb, ws_row, wbe_row, dst_sb, mtp):
                acc2 = ps.tile([P, 2 * TB], F32, tag="st", bufs=2, name="ka")
                for half in range(2):
                    mt = 2 * mtp + half
                    msl = slice(mt * P, (mt + 1) * P)
                    seg = acc2[:, half * TB:(half + 1) * TB]
                    for j in range(NC):
                        nc.tensor.matmul(seg, w_sb[:, j, msl], xt_t[:, j, :],
                                         start=(j == 0), stop=False)
                        if j == 2:
                            yield
                    nc.tensor.matmul(seg, ws_row[:, msl], mu[:],
                                     start=False, stop=(not use_be1))
                    if use_be1:
                        # be-term must survive the *1/sigma readout: rhs=sigma
                        nc.tensor.matmul(seg, wbe_row[:, msl], sig[:],
                                         start=False, stop=True)
                    yield
                for half in range(2):
                    mt = 2 * mtp + half
                    seg = acc2[:, half * TB:(half + 1) * TB]
                    nc.vector.tensor_mul(dst_sb[:, mt, tsl], seg, bcrs[:])

            for mtp in range(3):
                yield from proj_pair(wk_sb, nws_sb[1],
                                     wbe_sb[1] if use_be1 else None,
                                     kt_sb, mtp)
            for si in range(4):
                st = tb * 4 + si
                lsl = slice(si * P, (si + 1) * P)
                acc2 = ps.tile([P, 2 * TB], F32, tag="st", bufs=2, name="va")
                for half, fsl, off, w in ((0, slice(0, TB), 0, TB),
                                          (1, slice(TB, D), TB, D - TB)):
                    seg = acc2[:, off:off + w]
                    for j in range(NC):
                        nc.tensor.matmul(seg, xt_t[:, j, lsl],
                                         wv_sb[:, j, fsl],
                                         start=(j == 0), stop=False)
                        if j == 2:
                            yield
                    nc.tensor.matmul(seg, mu[:, lsl], nws_sb[2][:, fsl],
                                     start=False, stop=(not use_be1))
                    if use_be1:
                        nc.tensor.matmul(seg, sig[:, lsl], wbe_sb[2][:, fsl],
                                         start=False, stop=True)
                    yield
                for half, off, w in ((0, 0, TB), (1, TB, D - TB)):
                    src = acc2[:, off:off + w].rearrange(
                        "p (h e) -> p h e", e=64)
                    h0 = half * 8
                    nc.scalar.activation(
                        v_view[:, st, h0:h0 + w // 64, 0:64], src, AF.Copy,
                        scale=rcol[:, si:si + 1])
            if tb < 2:
                for mtp in range(3):
                    yield from proj_pair(wq_sb, nws_sb[0],
                                         wbe_sb[0] if use_be1 else None,
                                         qt_sb, mtp)

        def qkv_for_tb(*a):
            for _ in qkv_gen(*a):
                pass

        def attn_division(pend):
            # recip + broadcast + divide from the SBUF copies; deferred into
            # the NEXT mt's chunk stream so the bc matmuls never head-of-line
            # block the next mt's QKs
            yt_all, mt, yt_sbs = pend
            for ph in range(2):
                o = ph * 64
                yt_sb = yt_sbs[ph]
                rc = rows.tile([1, TB], F32R, tag="rc", bufs=2)
                nc.vector.reciprocal(rc[:], yt_sb[64:65, :])
                bc = ps.tile([64, TB], F32, tag="acc", bufs=2, name="abc")
                nc.tensor.matmul(bc[:], onesr_sb[:, 0:64],
                                 rc[:], start=True, stop=True)
                nc.vector.tensor_mul(yt_all[o:o + 64, mt, :],
                                     yt_sb[0:64, :], bc[:])

        def attn_mt(sl_i, yt_all, mt, fill, pend):
            chunks = SLOT_CHUNKS[sl_i]
            yt2 = [ps.tile([65, TB], F32, tag="yt", bufs=2,
                           name=f"yt_{sl_i}_{mt}_{ph}") for ph in range(2)]
            for ci, ch in enumerate(chunks):
                sb_idx = (0 if sl_i == 0 else 8) + ci
                di = ch - 4 * sl_i
                tri = 0 <= di < 4
                lo = 128 * di if tri else 0
                st2 = ps.tile([P, 2 * TB], F32, tag="st", bufs=2)
                qsub = slice(sl_i * TB + lo, (sl_i + 1) * TB)
                for ph in range(2):
                    o = ph * 64
                    nc.tensor.matmul(
                        st2[:, ph * TB + lo:(ph + 1) * TB],
                        kt_sb[o:o + 64, mt, ch * P:(ch + 1) * P],
                        qt_sb[o:o + 64, mt, qsub],
                        start=True, stop=True)
                e_tile = p_e.tile([P, 2 * TB], BF16, tag="e",
                                  name=f"e_{sl_i}_{mt}_{ci}")
                e_sb = e_tile[:]
                ev2 = e_sb.rearrange("p (two t) -> p two t", two=2)
                if lo:
                    ev = ev2[:, :, lo:]
                    sv = st2[:].rearrange("p (two t) -> p two t",
                                          two=2)[:, :, lo:]
                else:
                    ev, sv = e_sb, st2[:]
                nc.scalar.activation(
                    ev, sv, AF.Exp,
                    bias=bias_sb[:, sb_idx:sb_idx + 1],
                    scale=scale_sb[:, sb_idx:sb_idx + 1])
                if tri:
                    for ph in range(2):
                        nc.vector.tensor_mul(ev2[:, ph, lo:], ev2[:, ph, lo:],
                                             masks_sb[:, di, lo:])
                for ph in range(2):
                    h = 2 * mt + ph
                    nc.tensor.matmul(
                        yt2[ph][:, lo:], v_sb[:, ch, h * 65:(h + 1) * 65],
                        ev2[:, ph, lo:],
                        start=(ci == 0),
                        stop=(ci == len(chunks) - 1))
                if ci == 1 and pend is not None:
                    attn_division(pend)
                    pend = None
                if fill is not None:
                    next(fill, None)
            yt_sbs = []
            for ph in range(2):
                # copy [65,TB] to SBUF immediately: frees the PSUM bank so
                # the next mt's PV can start during the division
                yt_sb = work.tile([65, TB], F32R, tag="ydiv", bufs=3)
                nc.vector.tensor_copy(yt_sb[:], yt2[ph][:])
                yt_sbs.append(yt_sb)
            return (yt_all, mt, yt_sbs)


        with tc.tile_pool(name="p_xtr", bufs=3) as p_xtr:
            stats = {}
            xts = {}
            for tb in range(NTB):
                tsl = slice(tb * TB, (tb + 1) * TB)
                xt_t = p_xtr.tile([P, NC, TB], BF16, tag="xtr")
                if tb == 0:
                    # s1's lhsT and the first x chunk lead the DMA queue so
                    # the stats matmuls start as early as possible; the other
                    # consts ride behind the split x transfer
                    nc.sync.dma_start(onescb_sb[:], onescb_d)
                    nc.sync.dma_start(xt_t[:, 0:2, :], xtb_r[:, 0:2, tsl])
                    _early_const_dmas()
                    for jj in range(1, 3):
                        nc.sync.dma_start(xt_t[:, 2 * jj:2 * jj + 2, :],
                                          xtb_r[:, 2 * jj:2 * jj + 2, tsl])
                else:
                    nc.sync.dma_start(xt_t[:], xtb_r[:, :, tsl])
                if tb == 0:
                    nc.sync.dma_start(wk_sb[:],
                                      wkt_d.rearrange("(j p) m -> p j m", p=P))
                    nc.sync.dma_start(wv_sb[:],
                                      wvt_d.rearrange("(j p) m -> p j m", p=P))
                    nc.sync.dma_start(wq_sb[:],
                                      wqt_d.rearrange("(j p) m -> p j m", p=P))
                if tb == 1:
                    _late_const_dmas()
                xts[tb] = xt_t
                # emit qkv(tb-1) before stats(tb): the bulk PE work is ready
                # to run, so the in-order PE stream never parks on the
                # square/sum chain of the next block
                if tb > 0:
                    qkv_for_tb(tb - 1, xts[tb - 1][:], *stats[tb - 1])
                stats[tb] = ln1_stats(tb, xt_t)
            # slot0's first two mt's only need tb0/tb2 projections: run them
            # with qkv(3) threaded through as filler, so their exps use
            # phase-1's idle ACT while the last projection block feeds PE
            _mark(nc, "attn")
            gq = qkv_gen(NTB - 1, xts[NTB - 1][:], *stats[NTB - 1])
            pend0 = attn_mt(0, yt0, 0, gq, None)
            pend0 = attn_mt(0, yt0, 1, gq, pend0)
            for _ in gq:
                pass
        es_wqkv.close()

        # ---------------- Phase 3: attention (+ per-slot wo/LN2) -----------
        # Slot 1's chunk loop is exp(ACT)-bound with ~190ns/chunk of PE
        # slack, so slot 0's w_o projection, LN2 and the first MLP1 groups
        # are threaded through it as fine-grained fillers (one small PE
        # batch per attention chunk).
        _mark(nc, "attn")
        w1t_r = w1t_d.rearrange("g (j p) two f -> p g j two f", p=P)
        w2t_r = w2t_d.rearrange("(f p) two c -> p f two c", p=P)
        outt_r = outt_d.rearrange("(j p) t -> p j t", p=P)
        p_xp = ctx.enter_context(tc.tile_pool(name="p_xp", bufs=1))
        xp_sb = p_xp.tile([P, NC, 1024], F32R, tag="xp")
        p_xn2 = ctx.enter_context(tc.tile_pool(name="p_xn2", bufs=1))
        xn2_sb = p_xn2.tile([P, NC, 1024], F8, tag="xn2")
        p_h1a = ctx.enter_context(tc.tile_pool(name="p_h1a", bufs=1))
        p_wmlp = ctx.enter_context(tc.tile_pool(name="p_wmlp", bufs=2))
        h1s = {0: p_h1a.tile([P, NF, TB], F8, tag="h1a", name="h1a"),
               1: None}
        es_wo = ExitStack()
        p_wo = es_wo.enter_context(tc.tile_pool(name="p_wo", bufs=1))
        wo_sb = p_wo.tile([P, NC, D], BF16, tag="wo")
        nc.sync.dma_start(wo_sb[:], wo_d.rearrange("(j p) m -> p j m", p=P))
        xo0_sb = p_wo.tile([P, NC, TB], BF16, tag="xo", bufs=1, name="xo0")
        nc.sync.dma_start(xo0_sb[:], xtb_r[:, :, 0:TB])
        xo_hold = {0: xo0_sb}

        def gen_wo(sl_i, yt_all):
            _mark(nc, "wo")
            qsl = slice(sl_i * TB, (sl_i + 1) * TB)
            xo_t = xo_hold[sl_i]
            for ct in range(NC):
                ao = ps.tile([P, TB], F32, tag="acc", bufs=2, name="ao")
                for mc in range(NC):
                    nc.tensor.matmul(ao[:],
                                     wo_sb[:, mc, ct * P:(ct + 1) * P],
                                     yt_all[:, mc, :],
                                     start=(mc == 0), stop=(mc == NC - 1))
                    if mc % 2:
                        yield
                nc.vector.tensor_add(xp_sb[:, ct, qsl], xo_t[:, ct, :],
                                     ao[:])

        def gen_xo1():
            # slot1 residual load: reuses xo0's slot, so it must be emitted
            # only after all of gen_wo(0)'s reads
            xo1 = p_wo.tile([P, NC, TB], BF16, tag="xo", bufs=1, name="xo1")
            nc.sync.dma_start(xo1[:], xtb_r[:, :, TB:2 * TB])
            xo_hold[1] = xo1
            return
            yield

        def gen_ln2(sl_i):
            _mark(nc, "ln2")
            qsl = slice(sl_i * TB, (sl_i + 1) * TB)
            src = xp_sb[:, :, qsl]
            s1 = ps.tile([1, TB], F32, tag="acc", bufs=2, name="s1")
            s2 = ps.tile([1, TB], F32, tag="acc", bufs=2, name="s2")
            for j in range(NC):
                nc.tensor.matmul(s1[:], onesc_sb[:], src[:, j, :],
                                 start=(j == 0), stop=(j == NC - 1))
                if j % 2:
                    yield
            sqe = (nc.vector, nc.gpsimd)
            for j in range(NC):
                sq = work.tile([P, TB], F32R, tag="sq", bufs=2)
                sqe[j % 2].tensor_mul(sq[:], src[:, j, :], src[:, j, :])
                nc.tensor.matmul(s2[:], onesc_sb[:], sq[:],
                                 start=(j == 0), stop=(j == NC - 1))
                if j % 2:
                    yield
            mu2, r2, _ = _muvar(s1, s2)
            mur2 = rows.tile([1, TB], F32R, tag="mur", bufs=1)
            nc.vector.tensor_mul(mur2[:], mu2[:], r2[:])
            yield
            yield from gen_ln_normalize(src, xn2_sb[:, :, qsl], r2, mur2,
                                        g2_sb, be2_sb, g2r_sb)

        def gen_mlp1(groups, tbs, after_group=None):
            _mark(nc, "mlp1")
            for ft4 in groups:
                w1_t = p_wmlp.tile([P, NC, 2, 4 * P], F8, tag="w1", bufs=2)
                nc.sync.dma_start(w1_t[:], w1t_r[:, ft4])
                for sub in range(4):
                    ft = 4 * ft4 + sub
                    for tb in tbs:
                        tsl = slice(tb * TB, (tb + 1) * TB)
                        hp = ps.tile([P, TB], F32, tag="acc", bufs=2,
                                     name="hp")
                        for j in range(NC):
                            nc.tensor.matmul(
                                hp[:],
                                w1_t[:, j, :, sub * P:(sub + 1) * P],
                                xn2_sb[:, j, tsl].unsqueeze(1)
                                .broadcast_to([P, 2, TB]),
                                start=(j == 0), stop=(j == NC - 1),
                                perf_mode=DR)
                            if j % 3 == 2:
                                yield
                        # h1 = relu(psum + S_H1*b1) (S_H1 == S_W1 so scale=1)
                        if use_b1:
                            nc.scalar.activation(h1s[tb][:, ft, :], hp[:],
                                                 AF.Relu,
                                                 bias=b1_sb[:, ft:ft + 1],
                                                 scale=S_H1 / S_W1)
                        else:
                            nc.vector.tensor_scalar_max(h1s[tb][:, ft, :],
                                                        hp[:], 0.0)
                        yield
                if after_group is not None:
                    after_group(ft4)

        # slot 0 rest (mts 0-1 already ran inside phase 1)
        for mt in range(2, NC):
            pend0 = attn_mt(0, yt0, mt, None, pend0)
        # slot 1 with interleaved fillers
        import itertools as _it
        fill = _it.chain(gen_wo(0, yt0), gen_xo1(), gen_ln2(0),
                         gen_mlp1(range(0, 3), (0,)))
        for mt in range(NC):
            pend0 = attn_mt(1, yt1, mt, fill, pend0)
        attn_division(pend0)
        for _ in fill:
            pass

        es_masks.close()
        es_kqv.close()

        # prefetch the first tail-MLP1 w1 groups so their transfers run
        # under wo(slot1)/LN2(slot1)
        tail_items = [(3, 0), (4, 0), (5, 0)] + [(g, 1) for g in range(6)]
        w1_tiles = {}

        def _issue_w1(i):
            if i < len(tail_items):
                t = p_wmlp.tile([P, NC, 2, 4 * P], F8, tag="w1", bufs=2,
                                name=f"w1b_{i}")
                nc.sync.dma_start(t[:], w1t_r[:, tail_items[i][0]])
                w1_tiles[i] = t

        for i in range(2):
            _issue_w1(i)
        for _ in gen_wo(1, yt1):
            pass
        for _ in gen_ln2(1):
            pass

        es_wo.close()
        es_yt.close()

        # ---------------- Phase 6: MLP tail ----------------
        _mark(nc, "mlp")
        with tc.tile_pool(name="p_h1b", bufs=1) as p_h1b, \
             tc.tile_pool(name="p_w2", bufs=1) as p_w2, \
             tc.tile_pool(name="p_out", bufs=4) as p_out:
            h1s[1] = p_h1b.tile([P, NF, TB], F8, tag="h1b", name="h1b")
            w2_sb = p_w2.tile([P, NF, 2, D], F8, tag="w2full")
            # rest of MLP1 as a depth-3 w1-prefetch pipeline; w2 streams in
            # behind the w1 loads
            for i, (ft4, tb) in enumerate(tail_items):
                w1_t = w1_tiles.pop(i)
                tsl = slice(tb * TB, (tb + 1) * TB)
                for sub in range(4):
                    ft = 4 * ft4 + sub
                    hp = ps.tile([P, TB], F32, tag="acc", bufs=2, name="hp")
                    for j in range(NC):
                        nc.tensor.matmul(
                            hp[:], w1_t[:, j, :, sub * P:(sub + 1) * P],
                            xn2_sb[:, j, tsl].unsqueeze(1)
                            .broadcast_to([P, 2, TB]),
                            start=(j == 0), stop=(j == NC - 1),
                            perf_mode=DR)
                    nc.scalar.activation(h1s[tb][:, ft, :], hp[:], AF.Relu,
                                         bias=b1_sb[:, ft:ft + 1],
                                         scale=S_H1 / S_W1)
                _issue_w1(i + 2)
                if i < 4:
                    nc.sync.dma_start(w2_sb[:, 6 * i:6 * i + 6],
                                      w2t_r[:, 6 * i:6 * i + 6, :, :])
            # MLP2: uneven 4+2 column grouping so the last-finishing group
            # has only 4 output DMAs in the tail
            for cts in (range(0, 4), range(4, 6)):
                cts = list(cts)
                o2s = {}
                for idx, ct in enumerate(cts):
                    if idx < 2:
                        tag = ("acc", "yt")[idx]
                        for tb in range(2):
                            o2s[(ct, tb)] = ps.tile(
                                [P, TB], F32, tag=tag, bufs=2,
                                name=f"o2_{ct}_{tb}")
                    else:
                        stp = ps.tile([P, 2 * TB], F32, tag="st", bufs=2,
                                      name=f"o2st_{ct}")
                        o2s[(ct, 0)] = stp[:, 0:TB]
                        o2s[(ct, 1)] = stp[:, TB:2 * TB]
                # seed each accumulator with (b2/C_MLP) x ones
                for ct in cts:
                    for tb in range(2):
                        nc.tensor.matmul(o2s[(ct, tb)][:],
                                         b2s_sb[:, ct * P:(ct + 1) * P],
                                         ones512_sb[:],
                                         start=True, stop=False)
                for ft in range(NF):
                    for tb in range(2):
                        rhs = h1s[tb][:, ft, :].unsqueeze(1).broadcast_to(
                            [P, 2, TB])
                        for ct in cts:
                            nc.tensor.matmul(
                                o2s[(ct, tb)][:],
                                w2_sb[:, ft, :, ct * P:(ct + 1) * P],
                                rhs, start=False, stop=(ft == NF - 1),
                                perf_mode=DR)
                for tb in range(2):
                    tsl = slice(tb * TB, (tb + 1) * TB)
                    for ct in cts:
                        ot = p_out.tile([P, TB], F32, tag="ot",
                                        name=f"ot_{ct}_{tb}")
                        nc.vector.scalar_tensor_tensor(
                            ot[:], o2s[(ct, tb)][:], C_MLP,
                            xp_sb[:, ct, tsl], ALU.mult, ALU.add)
                        nc.sync.dma_start(outt_r[:, ct, tsl], ot[:])

    nc.compile()
    return nc


def _hilo(w, f8):
    """[..., n] -> [..., 2, n] fp8 (hi, residual-lo) planes."""
    hi = w.astype(f8)
    lo = (w - hi.astype(np.float32)).astype(f8)
    return np.ascontiguousarray(np.stack([hi, lo], axis=-2))


def _host_inputs(X, w_q, w_k, w_v, w_o, W1, b1, W2, b2, g1, be1, g2, be2):
    """Build the 8 per-core input dicts."""
    f32 = np.float32
    import ml_dtypes as _mld
    _f8 = _mld.float8_e4m3
    _bf = _mld.bfloat16
    g1v = np.asarray(g1, f32)
    be1v = np.asarray(be1, f32)
    # LN1 fold: g1 into the QKV weight columns; mean correction rows are the
    # negated column sums; optional be1 rows handle a nonzero LN1 shift
    wqg = np.asarray(w_q, f32).reshape(D, D) * g1v[None, :]
    wkg = np.asarray(w_k, f32).reshape(D, D) * g1v[None, :]
    wvg = np.asarray(w_v, f32).reshape(D, D) * g1v[None, :]
    wqt = np.ascontiguousarray(wqg.T.astype(_bf))
    wkt = np.ascontiguousarray(wkg.T.astype(_bf))
    wvt = np.ascontiguousarray(wvg.T.astype(_bf))
    nws = np.ascontiguousarray(np.stack(
        [-wqg.sum(axis=1), -wkg.sum(axis=1), -wvg.sum(axis=1)]).astype(f32))
    use_be1 = bool(np.any(be1v))
    wo = np.ascontiguousarray(np.asarray(w_o, f32).astype(_bf))
    w1t = None  # bf16, set below
    w2t = None  # bf16, set below
    onesr = np.ones((1, P), f32)
    onesc = np.ones((P, 1), f32)
    onescb = np.ones((P, 1), _bf)
    onesv = None  # set below after bf16 import
    # 4 canonical self-diagonal masks: mask[k][s, t] = (128k + s <= t)
    import ml_dtypes
    bf16 = ml_dtypes.bfloat16
    masks = np.zeros((4, P, TB), bf16)
    ar_s = np.arange(P)[:, None]
    ar_t = np.arange(TB)[None, :]
    for k in range(4):
        masks[k] = (128 * k + ar_s <= ar_t).astype(bf16)
    w1t = _hilo(np.asarray(W1, f32).T * S_W1, _f8)   # [D, 2, DFF]
    w1t = np.ascontiguousarray(
        w1t.reshape(D, 2, NF // 4, 4 * P).transpose(2, 0, 1, 3))
    w2t = _hilo(np.asarray(W2, f32).T * S_W2, _f8)

    # per-role exp scale/bias: 24 = 8 (slot0) + 16 (slot1) chunk positions
    sc = {}
    bi = {}
    for role in range(2):
        order = ROLE_ORDER[role]
        s = np.full((24,), 0.125, f32)
        b = np.zeros((24,), f32)
        for sl_i in range(2):
            own_blk = order[sl_i]
            for ci, ch in enumerate(SLOT_CHUNKS[sl_i]):
                idx = (0 if sl_i == 0 else 8) + ci
                pos = ch // 4           # permuted 512-block of this s-chunk
                blk = order[pos]
                if pos == sl_i or blk < own_blk:
                    pass                # diagonal (tri-masked) or past: live
                else:
                    s[idx] = 0.0        # future: dead
                    b[idx] = DEAD
        sc[role] = np.broadcast_to(s, (P, 24)).copy()
        bi[role] = np.broadcast_to(b, (P, 24)).copy()

    g2r = np.asarray(g2, f32).reshape(1, D)
    shared = dict(wqt=wqt, wkt=wkt, wvt=wvt, wo=wo, w1t=w1t, w2t=w2t,
                  g2r=g2r, nws=nws,
                  onesr=onesr, onesc=onesc, onescb=onescb, masks=masks,
                  g2v=np.asarray(g2, f32), be2v=np.asarray(be2, f32),
                  b1v=np.asarray(b1, f32) * S_H1,
                  b2s=np.asarray(b2, f32).reshape(1, D) * (S_H1 * S_W2),
                  ones512=np.ones((1, TB), f32))
    if use_be1:
        shared["wbe"] = np.ascontiguousarray(np.stack(
            [np.asarray(w, f32).reshape(D, D) @ be1v
             for w in (w_q, w_k, w_v)]).astype(f32))

    in_maps = []
    for core in range(8):
        role, b_idx = core // 4, core % 4
        order = ROLE_ORDER[role]
        xb = np.asarray(X[b_idx], f32)          # [T, D]
        xperm = np.concatenate([xb[o * TB:(o + 1) * TB] for o in order], axis=0)
        xt = np.ascontiguousarray(xperm.T)      # [D, T]
        m = dict(shared)
        m["xt"] = xt
        m["xtb"] = np.ascontiguousarray(xt.astype(_bf))
        m["scalein"] = sc[role]
        m["biasin"] = bi[role]
        in_maps.append(m)
    return in_maps


def _assemble(results, dtype):
    out = np.empty((B, T, D), dtype)
    for core in range(8):
        role, b_idx = core // 4, core % 4
        order = ROLE_ORDER[role]
        ot = results[core]["outt"]              # [D, 1024]
        for sl_i in range(2):
            blk = order[sl_i]
            out[b_idx, blk * TB:(blk + 1) * TB] = \
                ot[:, sl_i * TB:(sl_i + 1) * TB].T
    return out


def kernel(X, w_q, w_k, w_v, w_o, W1, b1, W2, b2, g1, be1, g2, be2,
           _want_results=False, _trace=False):
    use_be1 = bool(np.any(np.asarray(be1)))
    use_b1 = bool(np.any(np.asarray(b1)))
    key = ("nc", use_be1, use_b1)
    if key not in _cached:
        _cached[key] = _build_nc(use_be1=use_be1, use_b1=use_b1)
        _cached["nc"] = _cached[key]
    nc = _cached[key]
    in_maps = _host_inputs(X, w_q, w_k, w_v, w_o, W1, b1, W2, b2,
                           g1, be1, g2, be2)
    res = run_bass_kernel_spmd(nc, in_maps, core_ids=list(range(8)),
                               trace=_trace)
    out = _assemble(res.results, np.asarray(X).dtype)
    if _want_results:
        return out, res
    return out

